# revision 27
# baseline (speedup 1.0000x reference)
"""Multi-head attention (B=4, S=2048, D=1024, H=16, causal) on 8 TRN2 NeuronCores.

Sharding: core i handles batch i//2 and head-group i%2 (8 heads / 512 projection
columns). Each core computes a partial output projection over its 512 rows of Wo;
the host sums the two partials per batch and adds (bv @ Wo + bo). No device
collectives.

v2 dataflow (bf16 matmuls, fp32 softmax), all per core:
  - j-major attention: for each query block j (512 queries), all 8 heads attend;
    projection groups and the j-1 output-projection tiles are woven in as PE
    filler between score/AV chunks.
  - Causal staircase: for (h, j), k-tiles 0..4j run full-width (512 queries) in
    2-k-tile PSUM chunks; the last three diagonal k-tiles r=1..3 only cover the
    un-masked query windows (384/128/256 wide) packed into one 768-col PSUM
    chunk, skipping the 6 fully-masked 128x128 blocks per (h, j). One wide exp
    per chunk; the only element-level masking left is four [128,128] triangle
    multiplies per (h, j) against a single shared triangle tile.
  - No bias matmuls: bk cancels in softmax (per-query constant), bv commutes
    through softmax (rows sum to 1) and is added on host as bv @ Wo, bq rides
    the ACT-engine Q eviction as a per-partition bias.
  - Denominator rides the AV matmul as a 65th V column; normalization uses a
    DMA-reshaped reciprocal ([1,512] -> [128,4]) and a GPSIMD partition
    broadcast, as in v1.
"""

import sys

for _p in ("/opt/trn_rl_repo",):
    if _p not in sys.path:
        sys.path.insert(0, _p)

import numpy as np
import ml_dtypes

BF16 = ml_dtypes.bfloat16

B, S, D = 4, 2048, 1024
H, HD = 16, 64
HPC = H // 2          # heads per core: 8
DPC = D // 2          # projection cols per core: 512
NCORES = 8
SCALE = 1.0 / np.sqrt(np.float32(HD))
NKD_ = D // 128       # 8 contraction tiles for projections
NSB_ = S // 512       # 4 seq blocks

_compiled = None


def _build():
    import concourse.bacc as bacc
    import concourse.mybir as mybir
    import concourse.tile as tile

    f32 = mybir.dt.float32
    bf = mybir.dt.bfloat16
    Exp = mybir.ActivationFunctionType.Exp
    Copy = mybir.ActivationFunctionType.Copy
    Ident = mybir.ActivationFunctionType.Identity

    nc = bacc.Bacc("TRN2", target_bir_lowering=False, debug=False)

    # host pre-packs everything into [128, *] panels so each tensor (or each
    # seq-block wave of an x tensor) loads with ONE dma descriptor: the sync
    # engine spends ~610ns generating each descriptor, so the v1 layout's 126
    # input descriptors serialized ~77us of input streaming.
    xtq = nc.dram_tensor("xtq", [128, NSB_ * NKD_ * 512], bf, kind="ExternalInput")
    xtk = nc.dram_tensor("xtk", [128, NSB_ * NKD_ * 512], bf, kind="ExternalInput")
    xtv = nc.dram_tensor("xtv", [128, NSB_ * NKD_ * 512], bf, kind="ExternalInput")
    wq = nc.dram_tensor("wq", [128, NKD_ * DPC], bf, kind="ExternalInput")
    wk = nc.dram_tensor("wk", [128, NKD_ * DPC], bf, kind="ExternalInput")
    wv = nc.dram_tensor("wv", [128, NKD_ * DPC], bf, kind="ExternalInput")
    wo = nc.dram_tensor("wo", [128, 4 * D], bf, kind="ExternalInput")
    bqd = nc.dram_tensor("bqd", [128, 4], f32, kind="ExternalInput")
    trid = nc.dram_tensor("trid", [128, 128], bf, kind="ExternalInput")
    y = nc.dram_tensor("y", [S, D], bf, kind="ExternalOutput")

    NKD = NKD_            # 8 contraction tiles for projections
    NST = S // 128        # 16 seq tiles
    NSB = NSB_            # 4 seq blocks (query blocks j)
    NHP = HPC // 2        # 4 head pairs / 128-wide col groups

    # staircase packing for diagonal k-tiles r=1..3: (packed col offset, width)
    STAIR = {1: (0, 384), 3: (384, 128), 2: (512, 256)}

    with tile.TileContext(nc) as tc:
        with (
            tc.tile_pool(name="consts", bufs=1) as consts,
            tc.tile_pool(name="wqp", bufs=NKD) as wqp,
            tc.tile_pool(name="wkp", bufs=NKD) as wkp,
            tc.tile_pool(name="wvp", bufs=NKD) as wvp,
            tc.tile_pool(name="wop", bufs=4) as wop,
            tc.tile_pool(name="xt", bufs=1) as xtp,
            tc.tile_pool(name="qt", bufs=NHP) as qtp,
            tc.tile_pool(name="kt", bufs=NHP) as ktp,
            tc.tile_pool(name="vp", bufs=NST) as vpool,
            tc.tile_pool(name="ex", bufs=3) as expool,
            tc.tile_pool(name="ot", bufs=NHP) as otp,
            tc.tile_pool(name="ys", bufs=2) as ysp,
            tc.tile_pool(name="rc", bufs=2) as rcp,
            tc.tile_pool(name="ps", bufs=2, space="PSUM") as psp,
            tc.tile_pool(name="av", bufs=2, space="PSUM") as avp,
            tc.tile_pool(name="sc", bufs=2, space="PSUM") as scp,
        ):
            # constants (no PE warmup: HAM starts at full clock on hardware,
            # and a junk-matmul burst just trips the activity throttle early)
            trit = consts.tile([128, 128], bf, tag="tri")
            nc.sync.dma_start(trit[:], trid.ap()[:])
            bqt = consts.tile([128, 4], f32, tag="bqt")
            nc.sync.dma_start(bqt[:], bqd.ap()[:])
            onesc = consts.tile([1, 64], bf, tag="onesc")
            nc.gpsimd.memset(onesc[:], 1.0)

            # ---- input DMAs: one big descriptor per weight tensor / per
            # (x tensor, seq block) wave, all on the sync hw queue, in need
            # order. Consumers need the whole wave anyway (a projection group
            # reads all 8 kd slices of its sb).
            WW = NKD * 512    # 4096 cols per wave

            wvb = wvp.tile([128, WW], bf, name="wvb", tag="wv", bufs=1)
            nc.sync.dma_start(wvb[:], wv.ap()[:])
            wvt = [wvb[:, kd * 512:(kd + 1) * 512] for kd in range(NKD)]

            def x_wave(src_t, prefix, tag, sb):
                xt = xtp.tile([128, WW], bf, name=f"{prefix}{sb}", tag=tag,
                              bufs=1)
                nc.sync.dma_start(
                    xt[:], src_t.ap()[:, sb * WW:(sb + 1) * WW])
                return xt

            xtv_q = [[None] * NSB for _ in range(NKD)]
            xtq_q = [[None] * NSB for _ in range(NKD)]
            xtk_q = [[None] * NSB for _ in range(NKD)]

            def set_wave(qlist, big, sb):
                for kd in range(NKD):
                    qlist[kd][sb] = big[:, kd * 512:(kd + 1) * 512]

            # sb0 of everything first so V st0-3 / Q sb0 / K sb0 start early.
            # xtk waves share slots with older tiles whose readers finish well
            # before the k load's data is needed: xk0 is fresh, xk_sb (sb>=1)
            # reuses the xq_{sb-1} slot (read by the Q groups a phase earlier).
            set_wave(xtv_q, x_wave(xtv, "xv", "xv0", 0), 0)
            wqb = wqp.tile([128, WW], bf, name="wqb", tag="wq", bufs=1)
            nc.sync.dma_start(wqb[:], wq.ap()[:])
            wqt = [wqb[:, kd * 512:(kd + 1) * 512] for kd in range(NKD)]
            set_wave(xtq_q, x_wave(xtq, "xq", "xq0", 0), 0)
            wkb = wkp.tile([128, WW], bf, name="wkb", tag="wk", bufs=1)
            nc.sync.dma_start(wkb[:], wk.ap()[:])
            wkt = [wkb[:, kd * 512:(kd + 1) * 512] for kd in range(NKD)]
            set_wave(xtk_q, x_wave(xtk, "xk", "xk0", 0), 0)
            for sb in range(1, NSB):
                set_wave(xtv_q, x_wave(xtv, "xv", f"xv{sb}", sb), sb)
                set_wave(xtq_q, x_wave(xtq, "xq", f"xq{sb}", sb), sb)
                set_wave(xtk_q, x_wave(xtk, "xk", f"xq{sb - 1}", sb), sb)
                if sb == 1:
                    wob = wop.tile([128, 4 * D], bf, name="wob", tag="wo", bufs=1)
                    nc.sync.dma_start(wob[:], wo.ap()[:])
                    wot = [wob[:, hp * D:(hp + 1) * D] for hp in range(4)]

            # ---- V projection groups (natural layout, [8 heads x 65] incl.
            # ones column for the softmax denominator; no bias)
            vts = [vpool.tile([128, HPC * 65], bf, name=f"v{st}", tag="v")
                   for st in range(NST)]

            def v_group(st):
                def group():
                    ps = psp.tile([128, 512], f32, name="psv", tag="ps")
                    for kd in range(NKD):
                        nc.tensor.matmul(
                            ps[:],
                            xtv_q[kd][st // 4][:, (st % 4) * 128:(st % 4 + 1) * 128],
                            wvt[kd][:],
                            start=(kd == 0), stop=(kd == NKD - 1),
                        )
                    vt = vts[st]
                    v3 = vt[:].rearrange("p (h c) -> p h c", h=HPC, c=65)
                    nc.vector.tensor_copy(
                        v3[:, :, 0:64],
                        ps[:].rearrange("p (h c) -> p h c", h=HPC, c=64),
                    )
                    nc.gpsimd.memset(v3[:, :, 64:65], 1.0)
                return group

            # ---- QT / KT projection groups (transposed layout [cols, seq])
            qts, kts = [], []
            for pool, lst, nm in ((qtp, qts, "qt"), (ktp, kts, "kt")):
                for hp in range(NHP):
                    lst.append(pool.tile([128, S], bf, name=f"{nm}{hp}", tag=nm))

            def q_group(hp, sb):
                def group():
                    ps = psp.tile([128, 512], f32, name="psq", tag="ps")
                    for kd in range(NKD):
                        nc.tensor.matmul(
                            ps[:],
                            wqt[kd][:, hp * 128:(hp + 1) * 128],
                            xtq_q[kd][sb][:],
                            start=(kd == 0), stop=(kd == NKD - 1),
                        )
                    # bq rides the eviction as a per-partition ACT bias
                    nc.scalar.activation(
                        qts[hp][:, sb * 512:(sb + 1) * 512], ps[:], Ident,
                        bias=bqt[:, hp:hp + 1])
                return group

            def k_group(hp, sb):
                def group():
                    ps = psp.tile([128, 512], f32, name="psk", tag="ps")
                    for kd in range(NKD):
                        nc.tensor.matmul(
                            ps[:],
                            wkt[kd][:, hp * 128:(hp + 1) * 128],
                            xtk_q[kd][sb][:],
                            start=(kd == 0), stop=(kd == NKD - 1),
                        )
                    nc.vector.tensor_copy(kts[hp][:, sb * 512:(sb + 1) * 512], ps[:])
                return group

            ots = [otp.tile([128, S], bf, name=f"ot{i}", tag="ot") for i in range(NHP)]

            def yproj_group(st, eb):
                def group():
                    ps = psp.tile([128, 512], f32, name="psy", tag="ps")
                    for hp in range(NHP):
                        nc.tensor.matmul(
                            ps[:],
                            ots[hp][:, st * 128:(st + 1) * 128],
                            wot[hp][:, eb * 512:(eb + 1) * 512],
                            start=(hp == 0), stop=(hp == NHP - 1),
                        )
                    ys = ysp.tile([128, 512], bf, name="ys", tag="ys")
                    if (st + eb) % 2 == 0:
                        nc.vector.tensor_copy(ys[:], ps[:])
                    else:
                        nc.scalar.activation(ys[:], ps[:], Copy)
                    nc.sync.dma_start(
                        y.ap()[st * 128:(st + 1) * 128, eb * 512:(eb + 1) * 512],
                        ys[:],
                    )
                return group

            # ---- phase filler schedule (j-major attention). Within a phase,
            # fillers are ordered by DMA arrival: yproj (wo landed long ago)
            # first, then V/Q/K groups whose x quarters stream in during the
            # phase -- a popped filler whose input is still in flight stalls
            # the in-order PE queue.
            phase_fillers = {
                0: ([v_group(st) for st in range(4, 8)]
                    + [q_group(hp, 1) for hp in range(NHP)]
                    + [k_group(hp, 1) for hp in range(NHP)]),
                1: ([yproj_group(st, eb) for st in range(0, 4) for eb in range(2)]
                    + [v_group(st) for st in range(8, 12)]
                    + [q_group(hp, 2) for hp in range(NHP)]
                    + [k_group(hp, 2) for hp in range(NHP)]),
                2: ([yproj_group(st, eb) for st in range(4, 8) for eb in range(2)]
                    + [v_group(st) for st in range(12, 16)]
                    + [q_group(hp, 3) for hp in range(NHP)]
                    + [k_group(hp, 3) for hp in range(NHP)]),
                3: [yproj_group(st, eb) for st in range(8, 12) for eb in range(2)],
            }

            # ---- attention
            # pending[0] carries the delayed AV emit of the previous chunk --
            # including across attend/phase boundaries, so the PE never sits on
            # an exp tail: the next attend's scores interleave with it.
            pending = [None]

            def flush_pending():
                if pending[0] is not None:
                    pending[0]()
                    pending[0] = None

            def attend(h, j, pop):
                hp, sub = h // 2, h % 2
                base = sub * 64
                qt_h = qts[hp][base:base + 64, :]
                kt_h = kts[hp][base:base + 64, :]
                av = avp.tile([128, 512], f32, name="av", tag="av")

                # chunks: pairs of full-width k-tiles 0..4j-1, then k-tile 4j
                # (the r0 diagonal) alone, then the packed staircase r=1..3
                chunks = [list(range(c0, min(c0 + 2, 4 * j)))
                          for c0 in range(0, 4 * j, 2)]
                chunks.append([4 * j])       # r0, full width, triangle at col 0
                chunks.append("stair")

                first_av = [True]

                def make_av_full(ex, kts_c, stop):
                    def emit():
                        for r, kti in enumerate(kts_c):
                            nc.tensor.matmul(
                                av[0:65, :],
                                vts[kti][:, h * 65:(h + 1) * 65],
                                ex[:, r * 512:(r + 1) * 512],
                                start=(first_av[0] and r == 0),
                                stop=(stop and r == len(kts_c) - 1),
                            )
                        first_av[0] = False
                    return emit

                def make_av_stair(ex):
                    def emit():
                        last = list(STAIR)[-1]
                        for r, (off, wid) in STAIR.items():
                            nc.tensor.matmul(
                                av[0:65, 128 * r:512],
                                vts[4 * j + r][:, h * 65:(h + 1) * 65],
                                ex[:, off:off + wid],
                                start=False, stop=(r == last),
                            )
                        # normalize immediately after the closing AV. All on
                        # DVE + PE: a gpsimd reshape/broadcast chain here backs
                        # up the gpsimd queue and head-of-line blocks the DVE
                        # (mask muls) at phase boundaries. The reciprocal row
                        # is broadcast across partitions with a rank-1 matmul
                        # into the free rows 64:128 of the same av PSUM bank.
                        avs = ysp.tile([65, 512], f32, name="avs", tag="ys")
                        nc.vector.tensor_copy(avs[:], av[0:65, :])
                        rrow = rcp.tile([1, 512], bf, name="rrow", tag="rrow")
                        with nc.allow_low_precision(reason="bf16 1/denom row"):
                            nc.vector.reciprocal(rrow[:], avs[64:65, :])
                        nc.tensor.matmul(av[64:128, :], onesc[:], rrow[:],
                                         start=True, stop=True)
                        nc.vector.tensor_mul(
                            ots[hp][base:base + 64, j * 512:(j + 1) * 512],
                            avs[0:64, :],
                            av[64:128, :],
                        )
                    return emit

                for ci, ch in enumerate(chunks):
                    sc = scp.tile([128, 1024], f32, name="sc", tag="sc")
                    ex = expool.tile([128, 1024], bf, name="ex", tag="ex")
                    if ch == "stair":
                        for r, (off, wid) in STAIR.items():
                            nc.tensor.matmul(
                                sc[:, off:off + wid],
                                kt_h[:, (4 * j + r) * 128:(4 * j + r + 1) * 128],
                                qt_h[:, j * 512 + 128 * r:(j + 1) * 512],
                                start=True, stop=True,
                            )
                        nc.scalar.activation(ex[:, 0:768], sc[:, 0:768], Exp,
                                             scale=float(SCALE))
                        for r, (off, wid) in STAIR.items():
                            nc.vector.tensor_mul(
                                ex[:, off:off + 128],
                                ex[:, off:off + 128],
                                trit[:],
                            )
                        this_av = make_av_stair(ex)
                    else:
                        for r, kti in enumerate(ch):
                            nc.tensor.matmul(
                                sc[:, r * 512:(r + 1) * 512],
                                kt_h[:, kti * 128:(kti + 1) * 128],
                                qt_h[:, j * 512:(j + 1) * 512],
                                start=True, stop=True,
                            )
                        nw = len(ch) * 512
                        nc.scalar.activation(ex[:, 0:nw], sc[:, 0:nw], Exp,
                                             scale=float(SCALE))
                        if ch[-1] == 4 * j:   # r0 chunk: triangle at col 0
                            nc.vector.tensor_mul(
                                ex[:, (len(ch) - 1) * 512:(len(ch) - 1) * 512 + 128],
                                ex[:, (len(ch) - 1) * 512:(len(ch) - 1) * 512 + 128],
                                trit[:],
                            )
                        this_av = make_av_full(ex, ch, stop=False)
                    pop()
                    flush_pending()
                    pending[0] = this_av

            # upfront groups: V st0-3, Q sb0, K sb0
            for st in range(4):
                v_group(st)()
            for hp in range(NHP):
                q_group(hp, 0)()
            for hp in range(NHP):
                k_group(hp, 0)()

            for j in range(NSB):
                fillers = phase_fillers[j]
                nchunks = 8 * (j + 2)
                state = {"chunk": 0, "popped": 0}

                hold = 4 if j == NSB - 1 else 2

                def pop():
                    # hold fillers back so the phase-end drain always has PE
                    # work to cover the last attend's exp + normalize latency
                    # (more in the last phase: the final yproj tiles gate on
                    # the very last normalize)
                    state["chunk"] += 1
                    want = state["chunk"] * max(0, len(fillers) - hold) // nchunks
                    while state["popped"] < min(want, len(fillers)):
                        fillers[state["popped"]]()
                        state["popped"] += 1

                for h in range(HPC):
                    attend(h, j, pop)
                # drain remaining fillers; flush the last attend's AV +
                # normalize after the first one so its exp latency and the
                # normalize chain hide under filler matmuls
                if state["popped"] < len(fillers):
                    fillers[state["popped"]]()
                    state["popped"] += 1
                    flush_pending()
                while state["popped"] < len(fillers):
                    fillers[state["popped"]]()
                    state["popped"] += 1

            flush_pending()
            for st in range(12, 16):
                for eb in range(2):
                    yproj_group(st, eb)()

    nc.compile()
    return nc


def _tri_mask():
    # tri[k, q] = 1 iff key k <= query q within a 128x128 diagonal block
    return np.triu(np.ones((128, 128), np.float32)).astype(BF16)


def _pack_x(xb):
    # [S, D] -> [128, (sb kd) 512]: xt_r[p, (sb*8+kd)*512+c] = x.T[kd*128+p, sb*512+c]
    xT = np.ascontiguousarray(xb.T)                      # [1024, 2048]
    return np.ascontiguousarray(
        xT.reshape(NKD_, 128, NSB_, 512).transpose(1, 2, 0, 3).reshape(128, -1)
    ).astype(BF16)


def _pack_w(Wh):
    # [D, 512] -> [128, kd-major 4096]
    return np.ascontiguousarray(
        Wh.reshape(NKD_, 128, DPC).transpose(1, 0, 2).reshape(128, -1)
    ).astype(BF16)


def _shard_inputs(q_in, k_in, v_in, Wq, bq, Wk, bk, Wv, bv, Wo, bo):
    tri = _tri_mask()
    in_maps = []
    for core in range(NCORES):
        b, g = core // 2, core % 2
        cs = slice(g * DPC, (g + 1) * DPC)
        in_maps.append({
            "xtq": _pack_x(q_in[b]),
            "xtk": _pack_x(k_in[b]),
            "xtv": _pack_x(v_in[b]),
            "wq": _pack_w(Wq[:, cs]),
            "wk": _pack_w(Wk[:, cs]),
            "wv": _pack_w(Wv[:, cs]),
            "wo": np.ascontiguousarray(
                Wo[cs, :].reshape(4, 128, D).transpose(1, 0, 2).reshape(128, -1)
            ).astype(BF16),
            "bqd": np.ascontiguousarray(
                bq[cs].reshape(4, 128).T).astype(np.float32),
            "trid": tri,
        })
    return in_maps


def kernel(q_in, k_in, v_in, Wq, bq, Wk, bk, Wv, bv, Wo, bo, _trace=False):
    from concourse.bass_utils import run_bass_kernel_spmd

    global _compiled
    if _compiled is None:
        _compiled = _build()

    args = [np.asarray(a, np.float32) for a in
            (q_in, k_in, v_in, Wq, bq, Wk, bk, Wv, bv, Wo, bo)]
    in_maps = _shard_inputs(*args)
    res = run_bass_kernel_spmd(
        _compiled, in_maps, core_ids=list(range(NCORES)), trace=_trace,
    )
    # bk cancels in softmax; bv commutes through (rows sum to 1): fold on host
    tail = (args[8].astype(np.float32) @ args[9].astype(np.float32)
            + args[10].astype(np.float32))
    out = np.empty((B, S, D), np.float32)
    for b in range(B):
        out[b] = (res.results[2 * b]["y"].astype(np.float32)
                  + res.results[2 * b + 1]["y"].astype(np.float32) + tail)
    if _trace:
        kernel.last_results = res
    return out


# revision 33
# speedup vs baseline: 1.3628x; 1.3628x over previous
"""Multi-head attention (B=4, S=2048, D=1024, H=16, causal) on 8 TRN2 NeuronCores.

Sharding: core i handles batch i//2 and head-group i%2 (8 heads / 512 projection
columns). Each core computes a partial output projection over its 512 rows of Wo;
the host sums the two partials per batch and adds (bv @ Wo + bo). No device
collectives.

v2 dataflow (bf16 matmuls, fp32 softmax), all per core:
  - j-major attention: for each query block j (512 queries), all 8 heads attend;
    projection groups and the j-1 output-projection tiles are woven in as PE
    filler between score/AV chunks.
  - Causal staircase: for (h, j), k-tiles 0..4j run full-width (512 queries) in
    2-k-tile PSUM chunks; the last three diagonal k-tiles r=1..3 only cover the
    un-masked query windows (384/128/256 wide) packed into one 768-col PSUM
    chunk, skipping the 6 fully-masked 128x128 blocks per (h, j). One wide exp
    per chunk; the only element-level masking left is four [128,128] triangle
    multiplies per (h, j) against a single shared triangle tile.
  - No bias matmuls: bk cancels in softmax (per-query constant), bv commutes
    through softmax (rows sum to 1) and is added on host as bv @ Wo, bq rides
    the ACT-engine Q eviction as a per-partition bias.
  - Denominator rides the AV matmul as a 65th V column; normalization uses a
    DMA-reshaped reciprocal ([1,512] -> [128,4]) and a GPSIMD partition
    broadcast, as in v1.
"""

import sys

for _p in ("/opt/trn_rl_repo",):
    if _p not in sys.path:
        sys.path.insert(0, _p)

import numpy as np
import ml_dtypes

BF16 = ml_dtypes.bfloat16

B, S, D = 4, 2048, 1024
H, HD = 16, 64
HPC = H // 2          # heads per core: 8
DPC = D // 2          # projection cols per core: 512
NCORES = 8
SCALE = 1.0 / np.sqrt(np.float32(HD))
NKD_ = D // 128       # 8 contraction tiles for projections
NSB_ = S // 512       # 4 seq blocks

_compiled = None


def _build():
    import concourse.bacc as bacc
    import concourse.mybir as mybir
    import concourse.tile as tile

    f32 = mybir.dt.float32
    bf = mybir.dt.bfloat16
    Exp = mybir.ActivationFunctionType.Exp
    Copy = mybir.ActivationFunctionType.Copy
    Ident = mybir.ActivationFunctionType.Identity

    nc = bacc.Bacc("TRN2", target_bir_lowering=False, debug=False)

    # host pre-packs everything into [128, *] panels so each tensor (or each
    # seq-block wave of an x tensor) loads with ONE dma descriptor: the sync
    # engine spends ~610ns generating each descriptor, so the v1 layout's 126
    # input descriptors serialized ~77us of input streaming.
    xtq = nc.dram_tensor("xtq", [128, NSB_ * NKD_ * 512], bf, kind="ExternalInput")
    xtk = nc.dram_tensor("xtk", [128, NSB_ * NKD_ * 512], bf, kind="ExternalInput")
    xtv = nc.dram_tensor("xtv", [128, NSB_ * NKD_ * 512], bf, kind="ExternalInput")
    wq = nc.dram_tensor("wq", [128, NKD_ * DPC], bf, kind="ExternalInput")
    wk = nc.dram_tensor("wk", [128, NKD_ * DPC], bf, kind="ExternalInput")
    wv = nc.dram_tensor("wv", [128, NKD_ * DPC], bf, kind="ExternalInput")
    wo = nc.dram_tensor("wo", [128, 4 * D], bf, kind="ExternalInput")
    bqd = nc.dram_tensor("bqd", [128, 4], f32, kind="ExternalInput")
    trid = nc.dram_tensor("trid", [128, 128], bf, kind="ExternalInput")
    y = nc.dram_tensor("y", [S, D], bf, kind="ExternalOutput")

    NKD = NKD_            # 8 contraction tiles for projections
    NST = S // 128        # 16 seq tiles
    NSB = NSB_            # 4 seq blocks (query blocks j)
    NHP = HPC // 2        # 4 head pairs / 128-wide col groups

    # staircase packing for diagonal k-tiles r=1..3: (packed col offset, width)
    STAIR = {1: (0, 384), 3: (384, 128), 2: (512, 256)}

    with tile.TileContext(nc) as tc:
        with (
            tc.tile_pool(name="consts", bufs=1) as consts,
            tc.tile_pool(name="wqp", bufs=NKD) as wqp,
            tc.tile_pool(name="wkp", bufs=NKD) as wkp,
            tc.tile_pool(name="wvp", bufs=NKD) as wvp,
            tc.tile_pool(name="wop", bufs=4) as wop,
            tc.tile_pool(name="xt", bufs=1) as xtp,
            tc.tile_pool(name="qt", bufs=NHP) as qtp,
            tc.tile_pool(name="kt", bufs=NHP) as ktp,
            tc.tile_pool(name="vp", bufs=NST) as vpool,
            tc.tile_pool(name="ex", bufs=3) as expool,
            tc.tile_pool(name="ot", bufs=NHP) as otp,
            tc.tile_pool(name="ys", bufs=2) as ysp,
            tc.tile_pool(name="rb", bufs=2) as rbp,
            tc.tile_pool(name="rc", bufs=2) as rcp,
            tc.tile_pool(name="ps", bufs=2, space="PSUM") as psp,
            tc.tile_pool(name="av", bufs=2, space="PSUM") as avp,
            tc.tile_pool(name="sc", bufs=2, space="PSUM") as scp,
        ):
            # constants (no PE warmup: HAM starts at full clock on hardware,
            # and a junk-matmul burst just trips the activity throttle early)
            trit = consts.tile([128, 128], bf, tag="tri")
            nc.sync.dma_start(trit[:], trid.ap()[:])
            bqt = consts.tile([128, 4], f32, tag="bqt")
            nc.sync.dma_start(bqt[:], bqd.ap()[:])


            # ---- input DMAs: one big descriptor per weight tensor / per
            # (x tensor, seq block) wave, all on the sync hw queue, in need
            # order. Consumers need the whole wave anyway (a projection group
            # reads all 8 kd slices of its sb).
            WW = NKD * 512    # 4096 cols per wave

            wvb = wvp.tile([128, WW], bf, name="wvb", tag="wv", bufs=1)
            nc.sync.dma_start(wvb[:], wv.ap()[:])
            wvt = [wvb[:, kd * 512:(kd + 1) * 512] for kd in range(NKD)]

            def x_wave(src_t, prefix, tag, sb):
                xt = xtp.tile([128, WW], bf, name=f"{prefix}{sb}", tag=tag,
                              bufs=1)
                nc.sync.dma_start(
                    xt[:], src_t.ap()[:, sb * WW:(sb + 1) * WW])
                return xt

            xtv_q = [[None] * NSB for _ in range(NKD)]
            xtq_q = [[None] * NSB for _ in range(NKD)]
            xtk_q = [[None] * NSB for _ in range(NKD)]

            def set_wave(qlist, big, sb):
                for kd in range(NKD):
                    qlist[kd][sb] = big[:, kd * 512:(kd + 1) * 512]

            # sb0 of everything first so V st0-3 / Q sb0 / K sb0 start early.
            # xtk waves share slots with older tiles whose readers finish well
            # before the k load's data is needed: xk0 is fresh, xk_sb (sb>=1)
            # reuses the xq_{sb-1} slot (read by the Q groups a phase earlier).
            set_wave(xtv_q, x_wave(xtv, "xv", "xv0", 0), 0)
            wqb = wqp.tile([128, WW], bf, name="wqb", tag="wq", bufs=1)
            nc.sync.dma_start(wqb[:], wq.ap()[:])
            wqt = [wqb[:, kd * 512:(kd + 1) * 512] for kd in range(NKD)]
            set_wave(xtq_q, x_wave(xtq, "xq", "xq0", 0), 0)
            wkb = wkp.tile([128, WW], bf, name="wkb", tag="wk", bufs=1)
            nc.sync.dma_start(wkb[:], wk.ap()[:])
            wkt = [wkb[:, kd * 512:(kd + 1) * 512] for kd in range(NKD)]
            set_wave(xtk_q, x_wave(xtk, "xk", "xk0", 0), 0)
            for sb in range(1, NSB):
                set_wave(xtv_q, x_wave(xtv, "xv", f"xv{sb}", sb), sb)
                set_wave(xtq_q, x_wave(xtq, "xq", f"xq{sb}", sb), sb)
                set_wave(xtk_q, x_wave(xtk, "xk", f"xq{sb - 1}", sb), sb)
                if sb == 1:
                    wob = wop.tile([128, 4 * D], bf, name="wob", tag="wo", bufs=1)
                    nc.sync.dma_start(wob[:], wo.ap()[:])
                    wot = [wob[:, hp * D:(hp + 1) * D] for hp in range(4)]

            # ---- V projection groups (natural layout, [8 heads x 65] incl.
            # ones column for the softmax denominator; no bias)
            vts = [vpool.tile([128, HPC * 65], bf, name=f"v{st}", tag="v")
                   for st in range(NST)]

            def v_group(st):
                def group():
                    ps = psp.tile([128, 512], f32, name="psv", tag="ps")
                    for kd in range(NKD):
                        nc.tensor.matmul(
                            ps[:],
                            xtv_q[kd][st // 4][:, (st % 4) * 128:(st % 4 + 1) * 128],
                            wvt[kd][:],
                            start=(kd == 0), stop=(kd == NKD - 1),
                        )
                    vt = vts[st]
                    v3 = vt[:].rearrange("p (h c) -> p h c", h=HPC, c=65)
                    nc.vector.tensor_copy(
                        v3[:, :, 0:64],
                        ps[:].rearrange("p (h c) -> p h c", h=HPC, c=64),
                    )
                    nc.gpsimd.memset(v3[:, :, 64:65], 1.0)
                return group

            # ---- QT / KT projection groups (transposed layout [cols, seq])
            qts, kts = [], []
            for pool, lst, nm in ((qtp, qts, "qt"), (ktp, kts, "kt")):
                for hp in range(NHP):
                    lst.append(pool.tile([128, S], bf, name=f"{nm}{hp}", tag=nm))

            def q_group(hp, sb):
                def group():
                    ps = psp.tile([128, 512], f32, name="psq", tag="ps")
                    for kd in range(NKD):
                        nc.tensor.matmul(
                            ps[:],
                            wqt[kd][:, hp * 128:(hp + 1) * 128],
                            xtq_q[kd][sb][:],
                            start=(kd == 0), stop=(kd == NKD - 1),
                        )
                    # bq rides the eviction as a per-partition ACT bias
                    nc.scalar.activation(
                        qts[hp][:, sb * 512:(sb + 1) * 512], ps[:], Ident,
                        bias=bqt[:, hp:hp + 1])
                return group

            def k_group(hp, sb):
                def group():
                    ps = psp.tile([128, 512], f32, name="psk", tag="ps")
                    for kd in range(NKD):
                        nc.tensor.matmul(
                            ps[:],
                            wkt[kd][:, hp * 128:(hp + 1) * 128],
                            xtk_q[kd][sb][:],
                            start=(kd == 0), stop=(kd == NKD - 1),
                        )
                    nc.vector.tensor_copy(kts[hp][:, sb * 512:(sb + 1) * 512], ps[:])
                return group

            ots = [otp.tile([128, S], bf, name=f"ot{i}", tag="ot") for i in range(NHP)]

            def yproj_group(st, eb):
                def group():
                    ps = psp.tile([128, 512], f32, name="psy", tag="ps")
                    for hp in range(NHP):
                        nc.tensor.matmul(
                            ps[:],
                            ots[hp][:, st * 128:(st + 1) * 128],
                            wot[hp][:, eb * 512:(eb + 1) * 512],
                            start=(hp == 0), stop=(hp == NHP - 1),
                        )
                    ys = ysp.tile([128, 512], bf, name="ys", tag="ys")
                    if (st + eb) % 2 == 0:
                        nc.vector.tensor_copy(ys[:], ps[:])
                    else:
                        nc.scalar.activation(ys[:], ps[:], Copy)
                    nc.sync.dma_start(
                        y.ap()[st * 128:(st + 1) * 128, eb * 512:(eb + 1) * 512],
                        ys[:],
                    )
                return group

            # ---- phase filler schedule (j-major attention). Within a phase,
            # fillers are ordered by DMA arrival: yproj (wo landed long ago)
            # first, then V/Q/K groups whose x quarters stream in during the
            # phase -- a popped filler whose input is still in flight stalls
            # the in-order PE queue.
            phase_fillers = {
                0: ([v_group(st) for st in range(4, 8)]
                    + [q_group(hp, 1) for hp in range(NHP)]
                    + [k_group(hp, 1) for hp in range(NHP)]),
                1: ([yproj_group(st, eb) for st in range(0, 4) for eb in range(2)]
                    + [v_group(st) for st in range(8, 12)]
                    + [q_group(hp, 2) for hp in range(NHP)]
                    + [k_group(hp, 2) for hp in range(NHP)]),
                2: ([yproj_group(st, eb) for st in range(4, 8) for eb in range(2)]
                    + [v_group(st) for st in range(12, 16)]
                    + [q_group(hp, 3) for hp in range(NHP)]
                    + [k_group(hp, 3) for hp in range(NHP)]),
                3: [yproj_group(st, eb) for st in range(8, 12) for eb in range(2)],
            }

            # ---- attention
            # pending[0] carries the delayed AV emit of the previous chunk --
            # including across attend/phase boundaries, so the PE never sits on
            # an exp tail: the next attend's scores interleave with it.
            # norm_tail[0] defers the cross-engine half of a normalize chain
            # one further flush point, so by the time its DVE ops reach the
            # queue head their sync-DMA/gpsimd inputs have already landed
            # (otherwise they head-of-line block the mask multiplies).
            pending = [None]
            norm_tail = [None]

            def flush_pending():
                if norm_tail[0] is not None:
                    t = norm_tail[0]
                    norm_tail[0] = None
                    t()
                if pending[0] is not None:
                    p = pending[0]
                    pending[0] = None
                    p()

            def attend(h, j, pop):
                hp, sub = h // 2, h % 2
                base = sub * 64
                qt_h = qts[hp][base:base + 64, :]
                kt_h = kts[hp][base:base + 64, :]
                av = avp.tile([128, 512], f32, name="av", tag="av")

                # chunks: pairs of full-width k-tiles 0..4j-1, then k-tile 4j
                # (the r0 diagonal) alone, then the packed staircase r=1..3
                chunks = [list(range(c0, min(c0 + 2, 4 * j)))
                          for c0 in range(0, 4 * j, 2)]
                chunks.append([4 * j])       # r0, full width, triangle at col 0
                chunks.append("stair")

                first_av = [True]

                def make_av_full(ex, kts_c, stop):
                    def emit():
                        for r, kti in enumerate(kts_c):
                            nc.tensor.matmul(
                                av[0:65, :],
                                vts[kti][:, h * 65:(h + 1) * 65],
                                ex[:, r * 512:(r + 1) * 512],
                                start=(first_av[0] and r == 0),
                                stop=(stop and r == len(kts_c) - 1),
                            )
                        first_av[0] = False
                    return emit

                def make_av_stair(ex):
                    def emit():
                        last = list(STAIR)[-1]
                        for r, (off, wid) in STAIR.items():
                            nc.tensor.matmul(
                                av[0:65, 128 * r:512],
                                vts[4 * j + r][:, h * 65:(h + 1) * 65],
                                ex[:, off:off + wid],
                                start=False, stop=(r == last),
                            )
                        # normalize, first half: evict av and kick off the
                        # [1,512]->[128,4] reshape (sync queue -- it is nearly
                        # idle; DVE reciprocal is ~6.5ns/element so the narrow
                        # reshape is what makes it cheap). The cross-engine
                        # tail is deferred one flush point via norm_tail so
                        # its DVE ops never head-of-line block the mask muls.
                        avs = ysp.tile([65, 512], f32, name="avs", tag="avs")
                        nc.vector.tensor_copy(avs[:], av[0:65, :])
                        rsh = rcp.tile([128, 4], f32, name="rsh", tag="rsh")
                        nc.sync.dma_start(rsh[:], avs[64:65, :])

                        def tail():
                            rr_t = rcp.tile([128, 4], f32, name="rr", tag="rr")
                            nc.vector.reciprocal(rr_t[:], rsh[:])
                            rrow = rcp.tile([1, 512], f32, name="rrow",
                                            tag="rrow")
                            nc.sync.dma_start(rrow[:], rr_t[:])
                            rb = rbp.tile([64, 512], f32, name="rb", tag="rb")
                            nc.gpsimd.partition_broadcast(rb[:], rrow[:],
                                                          channels=64)
                            nc.vector.tensor_mul(
                                ots[hp][base:base + 64, j * 512:(j + 1) * 512],
                                avs[0:64, :],
                                rb[:],
                            )
                        norm_tail[0] = tail
                    return emit

                for ci, ch in enumerate(chunks):
                    sc = scp.tile([128, 1024], f32, name="sc", tag="sc")
                    ex = expool.tile([128, 1024], bf, name="ex", tag="ex")
                    if ch == "stair":
                        for r, (off, wid) in STAIR.items():
                            nc.tensor.matmul(
                                sc[:, off:off + wid],
                                kt_h[:, (4 * j + r) * 128:(4 * j + r + 1) * 128],
                                qt_h[:, j * 512 + 128 * r:(j + 1) * 512],
                                start=True, stop=True,
                            )
                        nc.scalar.activation(ex[:, 0:768], sc[:, 0:768], Exp,
                                             scale=float(SCALE))
                        for r, (off, wid) in STAIR.items():
                            nc.vector.tensor_mul(
                                ex[:, off:off + 128],
                                ex[:, off:off + 128],
                                trit[:],
                            )
                        this_av = make_av_stair(ex)
                    else:
                        for r, kti in enumerate(ch):
                            nc.tensor.matmul(
                                sc[:, r * 512:(r + 1) * 512],
                                kt_h[:, kti * 128:(kti + 1) * 128],
                                qt_h[:, j * 512:(j + 1) * 512],
                                start=True, stop=True,
                            )
                        nw = len(ch) * 512
                        nc.scalar.activation(ex[:, 0:nw], sc[:, 0:nw], Exp,
                                             scale=float(SCALE))
                        if ch[-1] == 4 * j:   # r0 chunk: triangle at col 0
                            nc.vector.tensor_mul(
                                ex[:, (len(ch) - 1) * 512:(len(ch) - 1) * 512 + 128],
                                ex[:, (len(ch) - 1) * 512:(len(ch) - 1) * 512 + 128],
                                trit[:],
                            )
                        this_av = make_av_full(ex, ch, stop=False)
                    pop()
                    flush_pending()
                    pending[0] = this_av

            # upfront groups: V st0-3, Q sb0, K sb0
            for st in range(4):
                v_group(st)()
            for hp in range(NHP):
                q_group(hp, 0)()
            for hp in range(NHP):
                k_group(hp, 0)()

            for j in range(NSB):
                fillers = phase_fillers[j]
                nchunks = 8 * (j + 2)
                state = {"chunk": 0, "popped": 0}

                hold = 4 if j == NSB - 1 else 2

                def pop():
                    # hold fillers back so the phase-end drain always has PE
                    # work to cover the last attend's exp + normalize latency
                    # (more in the last phase: the final yproj tiles gate on
                    # the very last normalize)
                    state["chunk"] += 1
                    want = state["chunk"] * max(0, len(fillers) - hold) // nchunks
                    while state["popped"] < min(want, len(fillers)):
                        fillers[state["popped"]]()
                        state["popped"] += 1

                for h in range(HPC):
                    attend(h, j, pop)
                # drain remaining fillers, interleaving the two flushes (last
                # attend's stair AV, then its normalize tail) between them so
                # exp latency and the normalize chain hide under filler work
                for _ in range(2):
                    if state["popped"] < len(fillers):
                        fillers[state["popped"]]()
                        state["popped"] += 1
                    flush_pending()
                while state["popped"] < len(fillers):
                    fillers[state["popped"]]()
                    state["popped"] += 1

            flush_pending()
            flush_pending()
            for st in range(12, 16):
                for eb in range(2):
                    yproj_group(st, eb)()

    nc.compile()
    return nc


def _tri_mask():
    # tri[k, q] = 1 iff key k <= query q within a 128x128 diagonal block
    return np.triu(np.ones((128, 128), np.float32)).astype(BF16)


def _pack_x(xb):
    # [S, D] -> [128, (sb kd) 512]: xt_r[p, (sb*8+kd)*512+c] = x.T[kd*128+p, sb*512+c]
    xT = np.ascontiguousarray(xb.T)                      # [1024, 2048]
    return np.ascontiguousarray(
        xT.reshape(NKD_, 128, NSB_, 512).transpose(1, 2, 0, 3).reshape(128, -1)
    ).astype(BF16)


def _pack_w(Wh):
    # [D, 512] -> [128, kd-major 4096]
    return np.ascontiguousarray(
        Wh.reshape(NKD_, 128, DPC).transpose(1, 0, 2).reshape(128, -1)
    ).astype(BF16)


def _shard_inputs(q_in, k_in, v_in, Wq, bq, Wk, bk, Wv, bv, Wo, bo):
    tri = _tri_mask()
    in_maps = []
    for core in range(NCORES):
        b, g = core // 2, core % 2
        cs = slice(g * DPC, (g + 1) * DPC)
        in_maps.append({
            "xtq": _pack_x(q_in[b]),
            "xtk": _pack_x(k_in[b]),
            "xtv": _pack_x(v_in[b]),
            "wq": _pack_w(Wq[:, cs]),
            "wk": _pack_w(Wk[:, cs]),
            "wv": _pack_w(Wv[:, cs]),
            "wo": np.ascontiguousarray(
                Wo[cs, :].reshape(4, 128, D).transpose(1, 0, 2).reshape(128, -1)
            ).astype(BF16),
            "bqd": np.ascontiguousarray(
                bq[cs].reshape(4, 128).T).astype(np.float32),
            "trid": tri,
        })
    return in_maps


def kernel(q_in, k_in, v_in, Wq, bq, Wk, bk, Wv, bv, Wo, bo, _trace=False):
    from concourse.bass_utils import run_bass_kernel_spmd

    global _compiled
    if _compiled is None:
        _compiled = _build()

    args = [np.asarray(a, np.float32) for a in
            (q_in, k_in, v_in, Wq, bq, Wk, bk, Wv, bv, Wo, bo)]
    in_maps = _shard_inputs(*args)
    res = run_bass_kernel_spmd(
        _compiled, in_maps, core_ids=list(range(NCORES)), trace=_trace,
    )
    # bk cancels in softmax; bv commutes through (rows sum to 1): fold on host
    tail = (args[8].astype(np.float32) @ args[9].astype(np.float32)
            + args[10].astype(np.float32))
    out = np.empty((B, S, D), np.float32)
    for b in range(B):
        out[b] = (res.results[2 * b]["y"].astype(np.float32)
                  + res.results[2 * b + 1]["y"].astype(np.float32) + tail)
    if _trace:
        kernel.last_results = res
    return out


# revision 37
# speedup vs baseline: 1.3702x; 1.0054x over previous
"""Multi-head attention (B=4, S=2048, D=1024, H=16, causal) on 8 TRN2 NeuronCores.

Sharding: core i handles batch i//2 and head-group i%2 (8 heads / 512 projection
columns). Each core computes a partial output projection over its 512 rows of Wo;
the host sums the two partials per batch and adds (bv @ Wo + bo). No device
collectives.

v2 dataflow (bf16 matmuls, fp32 softmax), all per core:
  - j-major attention: for each query block j (512 queries), all 8 heads attend;
    projection groups and the j-1 output-projection tiles are woven in as PE
    filler between score/AV chunks.
  - Causal staircase: for (h, j), k-tiles 0..4j run full-width (512 queries) in
    2-k-tile PSUM chunks; the last three diagonal k-tiles r=1..3 only cover the
    un-masked query windows (384/128/256 wide) packed into one 768-col PSUM
    chunk, skipping the 6 fully-masked 128x128 blocks per (h, j). One wide exp
    per chunk; the only element-level masking left is four [128,128] triangle
    multiplies per (h, j) against a single shared triangle tile.
  - No bias matmuls: bk cancels in softmax (per-query constant), bv commutes
    through softmax (rows sum to 1) and is added on host as bv @ Wo, bq rides
    the ACT-engine Q eviction as a per-partition bias.
  - Denominator rides the AV matmul as a 65th V column; normalization uses a
    DMA-reshaped reciprocal ([1,512] -> [128,4]) and a GPSIMD partition
    broadcast, as in v1.
"""

import sys

for _p in ("/opt/trn_rl_repo",):
    if _p not in sys.path:
        sys.path.insert(0, _p)

import numpy as np
import ml_dtypes

BF16 = ml_dtypes.bfloat16

B, S, D = 4, 2048, 1024
H, HD = 16, 64
HPC = H // 2          # heads per core: 8
DPC = D // 2          # projection cols per core: 512
NCORES = 8
SCALE = 1.0 / np.sqrt(np.float32(HD))
NKD_ = D // 128       # 8 contraction tiles for projections
NSB_ = S // 512       # 4 seq blocks

_compiled = None


def _build():
    import concourse.bacc as bacc
    import concourse.mybir as mybir
    import concourse.tile as tile

    f32 = mybir.dt.float32
    bf = mybir.dt.bfloat16
    Exp = mybir.ActivationFunctionType.Exp
    Copy = mybir.ActivationFunctionType.Copy
    Ident = mybir.ActivationFunctionType.Identity

    nc = bacc.Bacc("TRN2", target_bir_lowering=False, debug=False)

    # host pre-packs everything into [128, *] panels so each tensor (or each
    # seq-block wave of an x tensor) loads with ONE dma descriptor: the sync
    # engine spends ~610ns generating each descriptor, so the v1 layout's 126
    # input descriptors serialized ~77us of input streaming.
    xtq = nc.dram_tensor("xtq", [128, NSB_ * NKD_ * 512], bf, kind="ExternalInput")
    xtk = nc.dram_tensor("xtk", [128, NSB_ * NKD_ * 512], bf, kind="ExternalInput")
    xtv = nc.dram_tensor("xtv", [128, NSB_ * NKD_ * 512], bf, kind="ExternalInput")
    wq = nc.dram_tensor("wq", [128, NKD_ * DPC], bf, kind="ExternalInput")
    wk = nc.dram_tensor("wk", [128, NKD_ * DPC], bf, kind="ExternalInput")
    wv = nc.dram_tensor("wv", [128, NKD_ * DPC], bf, kind="ExternalInput")
    wo = nc.dram_tensor("wo", [128, 4 * D], bf, kind="ExternalInput")
    bqd = nc.dram_tensor("bqd", [128, 4], f32, kind="ExternalInput")
    trid = nc.dram_tensor("trid", [128, 128], bf, kind="ExternalInput")
    y = nc.dram_tensor("y", [S, D], bf, kind="ExternalOutput")

    NKD = NKD_            # 8 contraction tiles for projections
    NST = S // 128        # 16 seq tiles
    NSB = NSB_            # 4 seq blocks (query blocks j)
    NHP = HPC // 2        # 4 head pairs / 128-wide col groups

    # staircase packing for diagonal k-tiles r=1..3: (packed col offset, width)
    STAIR = {1: (0, 384), 3: (384, 128), 2: (512, 256)}

    with tile.TileContext(nc) as tc:
        with (
            tc.tile_pool(name="consts", bufs=1) as consts,
            tc.tile_pool(name="wqp", bufs=NKD) as wqp,
            tc.tile_pool(name="wkp", bufs=NKD) as wkp,
            tc.tile_pool(name="wvp", bufs=NKD) as wvp,
            tc.tile_pool(name="wop", bufs=4) as wop,
            tc.tile_pool(name="xt", bufs=1) as xtp,
            tc.tile_pool(name="qt", bufs=NHP) as qtp,
            tc.tile_pool(name="kt", bufs=NHP) as ktp,
            tc.tile_pool(name="vp", bufs=NST) as vpool,
            tc.tile_pool(name="ex", bufs=3) as expool,
            tc.tile_pool(name="ot", bufs=NHP) as otp,
            tc.tile_pool(name="ys", bufs=2) as ysp,
            tc.tile_pool(name="rb", bufs=2) as rbp,
            tc.tile_pool(name="rc", bufs=2) as rcp,
            tc.tile_pool(name="ps", bufs=2, space="PSUM") as psp,
            tc.tile_pool(name="av", bufs=2, space="PSUM") as avp,
            tc.tile_pool(name="sc", bufs=2, space="PSUM") as scp,
        ):
            # constants (no PE warmup: HAM starts at full clock on hardware,
            # and a junk-matmul burst just trips the activity throttle early)
            trit = consts.tile([128, 128], bf, tag="tri")
            nc.sync.dma_start(trit[:], trid.ap()[:])
            bqt = consts.tile([128, 4], f32, tag="bqt")
            nc.sync.dma_start(bqt[:], bqd.ap()[:])


            # ---- input DMAs: one big descriptor per weight tensor / per
            # (x tensor, seq block) wave, all on the sync hw queue, in need
            # order. Consumers need the whole wave anyway (a projection group
            # reads all 8 kd slices of its sb).
            WW = NKD * 512    # 4096 cols per wave

            HW_ = WW // 2

            def dma_halves(dst, src_ap):
                # two descriptors per first-phase panel: subtile deps let the
                # kd 0-3 matmuls start as soon as the first half lands
                nc.sync.dma_start(dst[:, 0:HW_], src_ap[:, 0:HW_])
                nc.sync.dma_start(dst[:, HW_:WW], src_ap[:, HW_:WW])

            wvb = wvp.tile([128, WW], bf, name="wvb", tag="wv", bufs=1)
            dma_halves(wvb, wv.ap())
            wvt = [wvb[:, kd * 512:(kd + 1) * 512] for kd in range(NKD)]

            def x_wave(src_t, prefix, tag, sb, halves=False):
                xt = xtp.tile([128, WW], bf, name=f"{prefix}{sb}", tag=tag,
                              bufs=1)
                src = src_t.ap()[:, sb * WW:(sb + 1) * WW]
                if halves:
                    dma_halves(xt, src)
                else:
                    nc.sync.dma_start(xt[:], src)
                return xt

            xtv_q = [[None] * NSB for _ in range(NKD)]
            xtq_q = [[None] * NSB for _ in range(NKD)]
            xtk_q = [[None] * NSB for _ in range(NKD)]

            def set_wave(qlist, big, sb):
                for kd in range(NKD):
                    qlist[kd][sb] = big[:, kd * 512:(kd + 1) * 512]

            # sb0 of everything first so V st0-3 / Q sb0 / K sb0 start early.
            # xtk waves share slots with older tiles whose readers finish well
            # before the k load's data is needed: xk0 is fresh, xk_sb (sb>=1)
            # reuses the xq_{sb-1} slot (read by the Q groups a phase earlier).
            set_wave(xtv_q, x_wave(xtv, "xv", "xv0", 0, halves=True), 0)
            wqb = wqp.tile([128, WW], bf, name="wqb", tag="wq", bufs=1)
            dma_halves(wqb, wq.ap())
            wqt = [wqb[:, kd * 512:(kd + 1) * 512] for kd in range(NKD)]
            set_wave(xtq_q, x_wave(xtq, "xq", "xq0", 0, halves=True), 0)
            wkb = wkp.tile([128, WW], bf, name="wkb", tag="wk", bufs=1)
            dma_halves(wkb, wk.ap())
            wkt = [wkb[:, kd * 512:(kd + 1) * 512] for kd in range(NKD)]
            set_wave(xtk_q, x_wave(xtk, "xk", "xk0", 0, halves=True), 0)
            # xk1 reuses the xq0 slot, xk3 reuses xq2; xk2 gets its own slot
            # (the xq1 readers it would wait on finish too late in phase 1)
            xk_tag = {1: "xq0", 2: "xk2", 3: "xq2"}
            for sb in range(1, NSB):
                set_wave(xtv_q, x_wave(xtv, "xv", f"xv{sb}", sb), sb)
                set_wave(xtq_q, x_wave(xtq, "xq", f"xq{sb}", sb), sb)
                set_wave(xtk_q, x_wave(xtk, "xk", xk_tag[sb], sb), sb)
                if sb == 1:
                    wob = wop.tile([128, 4 * D], bf, name="wob", tag="wo", bufs=1)
                    nc.sync.dma_start(wob[:], wo.ap()[:])
                    wot = [wob[:, hp * D:(hp + 1) * D] for hp in range(4)]

            # ---- V projection groups (natural layout, [8 heads x 65] incl.
            # ones column for the softmax denominator; no bias)
            vts = [vpool.tile([128, HPC * 65], bf, name=f"v{st}", tag="v")
                   for st in range(NST)]

            def v_group(st):
                def group():
                    ps = psp.tile([128, 512], f32, name="psv", tag="ps")
                    for kd in range(NKD):
                        nc.tensor.matmul(
                            ps[:],
                            xtv_q[kd][st // 4][:, (st % 4) * 128:(st % 4 + 1) * 128],
                            wvt[kd][:],
                            start=(kd == 0), stop=(kd == NKD - 1),
                        )
                    vt = vts[st]
                    v3 = vt[:].rearrange("p (h c) -> p h c", h=HPC, c=65)
                    nc.vector.tensor_copy(
                        v3[:, :, 0:64],
                        ps[:].rearrange("p (h c) -> p h c", h=HPC, c=64),
                    )
                    nc.gpsimd.memset(v3[:, :, 64:65], 1.0)
                return group

            # ---- QT / KT projection groups (transposed layout [cols, seq])
            qts, kts = [], []
            for pool, lst, nm in ((qtp, qts, "qt"), (ktp, kts, "kt")):
                for hp in range(NHP):
                    lst.append(pool.tile([128, S], bf, name=f"{nm}{hp}", tag=nm))

            def q_group(hp, sb):
                def group():
                    ps = psp.tile([128, 512], f32, name="psq", tag="ps")
                    for kd in range(NKD):
                        nc.tensor.matmul(
                            ps[:],
                            wqt[kd][:, hp * 128:(hp + 1) * 128],
                            xtq_q[kd][sb][:],
                            start=(kd == 0), stop=(kd == NKD - 1),
                        )
                    # bq rides the eviction as a per-partition ACT bias
                    nc.scalar.activation(
                        qts[hp][:, sb * 512:(sb + 1) * 512], ps[:], Ident,
                        bias=bqt[:, hp:hp + 1])
                return group

            def k_group(hp, sb):
                def group():
                    ps = psp.tile([128, 512], f32, name="psk", tag="ps")
                    for kd in range(NKD):
                        nc.tensor.matmul(
                            ps[:],
                            wkt[kd][:, hp * 128:(hp + 1) * 128],
                            xtk_q[kd][sb][:],
                            start=(kd == 0), stop=(kd == NKD - 1),
                        )
                    nc.vector.tensor_copy(kts[hp][:, sb * 512:(sb + 1) * 512], ps[:])
                return group

            ots = [otp.tile([128, S], bf, name=f"ot{i}", tag="ot") for i in range(NHP)]

            def yproj_group(st, eb):
                def group():
                    ps = psp.tile([128, 512], f32, name="psy", tag="ps")
                    for hp in range(NHP):
                        nc.tensor.matmul(
                            ps[:],
                            ots[hp][:, st * 128:(st + 1) * 128],
                            wot[hp][:, eb * 512:(eb + 1) * 512],
                            start=(hp == 0), stop=(hp == NHP - 1),
                        )
                    ys = ysp.tile([128, 512], bf, name="ys", tag="ys")
                    if (st + eb) % 2 == 0:
                        nc.vector.tensor_copy(ys[:], ps[:])
                    else:
                        nc.scalar.activation(ys[:], ps[:], Copy)
                    nc.sync.dma_start(
                        y.ap()[st * 128:(st + 1) * 128, eb * 512:(eb + 1) * 512],
                        ys[:],
                    )
                return group

            # ---- phase filler schedule (j-major attention). Within a phase,
            # fillers are ordered by DMA arrival: yproj (wo landed long ago)
            # first, then V/Q/K groups whose x quarters stream in during the
            # phase -- a popped filler whose input is still in flight stalls
            # the in-order PE queue.
            phase_fillers = {
                0: ([v_group(st) for st in range(4, 8)]
                    + [q_group(hp, 1) for hp in range(NHP)]
                    + [k_group(hp, 1) for hp in range(NHP)]),
                1: ([v_group(st) for st in range(8, 12)]
                    + [yproj_group(st, eb) for st in range(0, 4) for eb in range(2)]
                    + [q_group(hp, 2) for hp in range(NHP)]
                    + [k_group(hp, 2) for hp in range(NHP)]),
                2: ([v_group(st) for st in range(12, 16)]
                    + [yproj_group(st, eb) for st in range(4, 8) for eb in range(2)]
                    + [q_group(hp, 3) for hp in range(NHP)]
                    + [k_group(hp, 3) for hp in range(NHP)]),
                3: [yproj_group(st, eb) for st in range(8, 12) for eb in range(2)],
            }

            # ---- attention
            # pending[0] carries the delayed AV emit of the previous chunk --
            # including across attend/phase boundaries, so the PE never sits on
            # an exp tail: the next attend's scores interleave with it.
            # norm_tail[0] defers the cross-engine half of a normalize chain
            # one further flush point, so by the time its DVE ops reach the
            # queue head their sync-DMA/gpsimd inputs have already landed
            # (otherwise they head-of-line block the mask multiplies).
            pending = [None]
            norm_tail = [None]

            def flush_pending():
                if norm_tail[0] is not None:
                    t = norm_tail[0]
                    norm_tail[0] = None
                    t()
                if pending[0] is not None:
                    p = pending[0]
                    pending[0] = None
                    p()

            def attend(h, j, pop):
                hp, sub = h // 2, h % 2
                base = sub * 64
                qt_h = qts[hp][base:base + 64, :]
                kt_h = kts[hp][base:base + 64, :]
                av = avp.tile([128, 512], f32, name="av", tag="av")

                # chunks: pairs of full-width k-tiles 0..4j-1, then k-tile 4j
                # (the r0 diagonal) alone, then the packed staircase r=1..3
                chunks = [list(range(c0, min(c0 + 2, 4 * j)))
                          for c0 in range(0, 4 * j, 2)]
                chunks.append([4 * j])       # r0, full width, triangle at col 0
                chunks.append("stair")

                first_av = [True]

                def make_av_full(ex, kts_c, stop):
                    def emit():
                        for r, kti in enumerate(kts_c):
                            nc.tensor.matmul(
                                av[0:65, :],
                                vts[kti][:, h * 65:(h + 1) * 65],
                                ex[:, r * 512:(r + 1) * 512],
                                start=(first_av[0] and r == 0),
                                stop=(stop and r == len(kts_c) - 1),
                            )
                        first_av[0] = False
                    return emit

                def make_av_stair(ex):
                    def emit():
                        last = list(STAIR)[-1]
                        for r, (off, wid) in STAIR.items():
                            nc.tensor.matmul(
                                av[0:65, 128 * r:512],
                                vts[4 * j + r][:, h * 65:(h + 1) * 65],
                                ex[:, off:off + wid],
                                start=False, stop=(r == last),
                            )
                        # normalize, first half: evict av and kick off the
                        # [1,512]->[128,4] reshape (sync queue -- it is nearly
                        # idle; DVE reciprocal is ~6.5ns/element so the narrow
                        # reshape is what makes it cheap). The cross-engine
                        # tail is deferred one flush point via norm_tail so
                        # its DVE ops never head-of-line block the mask muls.
                        avs = ysp.tile([65, 512], f32, name="avs", tag="avs")
                        nc.vector.tensor_copy(avs[:], av[0:65, :])
                        rsh = rcp.tile([128, 4], f32, name="rsh", tag="rsh")
                        nc.sync.dma_start(rsh[:], avs[64:65, :])

                        def tail():
                            rr_t = rcp.tile([128, 4], f32, name="rr", tag="rr")
                            nc.vector.reciprocal(rr_t[:], rsh[:])
                            rrow = rcp.tile([1, 512], f32, name="rrow",
                                            tag="rrow")
                            nc.sync.dma_start(rrow[:], rr_t[:])
                            rb = rbp.tile([64, 512], f32, name="rb", tag="rb")
                            nc.gpsimd.partition_broadcast(rb[:], rrow[:],
                                                          channels=64)
                            nc.vector.tensor_mul(
                                ots[hp][base:base + 64, j * 512:(j + 1) * 512],
                                avs[0:64, :],
                                rb[:],
                            )
                        norm_tail[0] = tail
                    return emit

                for ci, ch in enumerate(chunks):
                    sc = scp.tile([128, 1024], f32, name="sc", tag="sc")
                    ex = expool.tile([128, 1024], bf, name="ex", tag="ex")
                    if ch == "stair":
                        for r, (off, wid) in STAIR.items():
                            nc.tensor.matmul(
                                sc[:, off:off + wid],
                                kt_h[:, (4 * j + r) * 128:(4 * j + r + 1) * 128],
                                qt_h[:, j * 512 + 128 * r:(j + 1) * 512],
                                start=True, stop=True,
                            )
                        nc.scalar.activation(ex[:, 0:768], sc[:, 0:768], Exp,
                                             scale=float(SCALE))
                        for r, (off, wid) in STAIR.items():
                            nc.vector.tensor_mul(
                                ex[:, off:off + 128],
                                ex[:, off:off + 128],
                                trit[:],
                            )
                        this_av = make_av_stair(ex)
                    else:
                        for r, kti in enumerate(ch):
                            nc.tensor.matmul(
                                sc[:, r * 512:(r + 1) * 512],
                                kt_h[:, kti * 128:(kti + 1) * 128],
                                qt_h[:, j * 512:(j + 1) * 512],
                                start=True, stop=True,
                            )
                        nw = len(ch) * 512
                        nc.scalar.activation(ex[:, 0:nw], sc[:, 0:nw], Exp,
                                             scale=float(SCALE))
                        if ch[-1] == 4 * j:   # r0 chunk: triangle at col 0
                            nc.vector.tensor_mul(
                                ex[:, (len(ch) - 1) * 512:(len(ch) - 1) * 512 + 128],
                                ex[:, (len(ch) - 1) * 512:(len(ch) - 1) * 512 + 128],
                                trit[:],
                            )
                        this_av = make_av_full(ex, ch, stop=False)
                    pop()
                    flush_pending()
                    pending[0] = this_av

            # upfront groups: V st0-3, Q sb0, K sb0
            for st in range(4):
                v_group(st)()
            for hp in range(NHP):
                q_group(hp, 0)()
            for hp in range(NHP):
                k_group(hp, 0)()

            for j in range(NSB):
                fillers = phase_fillers[j]
                nchunks = 8 * (j + 2)
                state = {"chunk": 0, "popped": 0}

                # last phase: hold ALL fillers for the drain -- per-chunk PE
                # work exceeds ACT there, so attends self-sustain, and the
                # final yproj tiles gate on the very last normalize chain
                hold = len(fillers) if j == NSB - 1 else 2

                def pop():
                    # hold fillers back so the phase-end drain always has PE
                    # work to cover the last attend's exp + normalize latency
                    # (more in the last phase: the final yproj tiles gate on
                    # the very last normalize)
                    state["chunk"] += 1
                    want = state["chunk"] * max(0, len(fillers) - hold) // nchunks
                    while state["popped"] < min(want, len(fillers)):
                        fillers[state["popped"]]()
                        state["popped"] += 1

                for h in range(HPC):
                    attend(h, j, pop)
                # drain remaining fillers, interleaving the two flushes (last
                # attend's stair AV, then its normalize tail) between them so
                # exp latency and the normalize chain hide under filler work
                for _ in range(2):
                    if state["popped"] < len(fillers):
                        fillers[state["popped"]]()
                        state["popped"] += 1
                    flush_pending()
                while state["popped"] < len(fillers):
                    fillers[state["popped"]]()
                    state["popped"] += 1

            flush_pending()
            flush_pending()
            for st in range(12, 16):
                for eb in range(2):
                    yproj_group(st, eb)()

    nc.compile()
    return nc


def _tri_mask():
    # tri[k, q] = 1 iff key k <= query q within a 128x128 diagonal block
    return np.triu(np.ones((128, 128), np.float32)).astype(BF16)


def _pack_x(xb):
    # [S, D] -> [128, (sb kd) 512]: xt_r[p, (sb*8+kd)*512+c] = x.T[kd*128+p, sb*512+c]
    xT = np.ascontiguousarray(xb.T)                      # [1024, 2048]
    return np.ascontiguousarray(
        xT.reshape(NKD_, 128, NSB_, 512).transpose(1, 2, 0, 3).reshape(128, -1)
    ).astype(BF16)


def _pack_w(Wh):
    # [D, 512] -> [128, kd-major 4096]
    return np.ascontiguousarray(
        Wh.reshape(NKD_, 128, DPC).transpose(1, 0, 2).reshape(128, -1)
    ).astype(BF16)


def _shard_inputs(q_in, k_in, v_in, Wq, bq, Wk, bk, Wv, bv, Wo, bo):
    tri = _tri_mask()
    in_maps = []
    for core in range(NCORES):
        b, g = core // 2, core % 2
        cs = slice(g * DPC, (g + 1) * DPC)
        in_maps.append({
            "xtq": _pack_x(q_in[b]),
            "xtk": _pack_x(k_in[b]),
            "xtv": _pack_x(v_in[b]),
            "wq": _pack_w(Wq[:, cs]),
            "wk": _pack_w(Wk[:, cs]),
            "wv": _pack_w(Wv[:, cs]),
            "wo": np.ascontiguousarray(
                Wo[cs, :].reshape(4, 128, D).transpose(1, 0, 2).reshape(128, -1)
            ).astype(BF16),
            "bqd": np.ascontiguousarray(
                bq[cs].reshape(4, 128).T).astype(np.float32),
            "trid": tri,
        })
    return in_maps


def kernel(q_in, k_in, v_in, Wq, bq, Wk, bk, Wv, bv, Wo, bo, _trace=False):
    from concourse.bass_utils import run_bass_kernel_spmd

    global _compiled
    if _compiled is None:
        _compiled = _build()

    args = [np.asarray(a, np.float32) for a in
            (q_in, k_in, v_in, Wq, bq, Wk, bk, Wv, bv, Wo, bo)]
    in_maps = _shard_inputs(*args)
    res = run_bass_kernel_spmd(
        _compiled, in_maps, core_ids=list(range(NCORES)), trace=_trace,
    )
    # bk cancels in softmax; bv commutes through (rows sum to 1): fold on host
    tail = (args[8].astype(np.float32) @ args[9].astype(np.float32)
            + args[10].astype(np.float32))
    out = np.empty((B, S, D), np.float32)
    for b in range(B):
        out[b] = (res.results[2 * b]["y"].astype(np.float32)
                  + res.results[2 * b + 1]["y"].astype(np.float32) + tail)
    if _trace:
        kernel.last_results = res
    return out


# revision 44
# speedup vs baseline: 1.3825x; 1.0090x over previous
"""Multi-head attention (B=4, S=2048, D=1024, H=16, causal) on 8 TRN2 NeuronCores.

Sharding: core i handles batch i//2 and head-group i%2 (8 heads / 512 projection
columns). Each core computes a partial output projection over its 512 rows of Wo;
the host sums the two partials per batch and adds (bv @ Wo + bo). No device
collectives.

v2 dataflow (bf16 matmuls, fp32 softmax), all per core:
  - j-major attention: for each query block j (512 queries), all 8 heads attend;
    projection groups and the j-1 output-projection tiles are woven in as PE
    filler between score/AV chunks.
  - Causal staircase: for (h, j), k-tiles 0..4j run full-width (512 queries) in
    2-k-tile PSUM chunks; the last three diagonal k-tiles r=1..3 only cover the
    un-masked query windows (384/128/256 wide) packed into one 768-col PSUM
    chunk, skipping the 6 fully-masked 128x128 blocks per (h, j). One wide exp
    per chunk; the only element-level masking left is four [128,128] triangle
    multiplies per (h, j) against a single shared triangle tile.
  - No bias matmuls: bk cancels in softmax (per-query constant), bv commutes
    through softmax (rows sum to 1) and is added on host as bv @ Wo, bq rides
    the ACT-engine Q eviction as a per-partition bias.
  - Denominator rides the AV matmul as a 65th V column; normalization uses a
    DMA-reshaped reciprocal ([1,512] -> [128,4]) and a GPSIMD partition
    broadcast, as in v1.
"""

import sys

for _p in ("/opt/trn_rl_repo",):
    if _p not in sys.path:
        sys.path.insert(0, _p)

import numpy as np
import ml_dtypes

BF16 = ml_dtypes.bfloat16

B, S, D = 4, 2048, 1024
H, HD = 16, 64
HPC = H // 2          # heads per core: 8
DPC = D // 2          # projection cols per core: 512
NCORES = 8
SCALE = 1.0 / np.sqrt(np.float32(HD))
NKD_ = D // 128       # 8 contraction tiles for projections
NSB_ = S // 512       # 4 seq blocks

_compiled = None


def _build():
    import concourse.bacc as bacc
    import concourse.mybir as mybir
    import concourse.tile as tile

    f32 = mybir.dt.float32
    bf = mybir.dt.bfloat16
    Exp = mybir.ActivationFunctionType.Exp
    Copy = mybir.ActivationFunctionType.Copy
    Ident = mybir.ActivationFunctionType.Identity

    nc = bacc.Bacc("TRN2", target_bir_lowering=False, debug=False)

    # host pre-packs everything into [128, *] panels so each tensor (or each
    # seq-block wave of an x tensor) loads with ONE dma descriptor: the sync
    # engine spends ~610ns generating each descriptor, so the v1 layout's 126
    # input descriptors serialized ~77us of input streaming.
    xtq = nc.dram_tensor("xtq", [128, NSB_ * NKD_ * 512], bf, kind="ExternalInput")
    xtk = nc.dram_tensor("xtk", [128, NSB_ * NKD_ * 512], bf, kind="ExternalInput")
    xtv = nc.dram_tensor("xtv", [128, NSB_ * NKD_ * 512], bf, kind="ExternalInput")
    wq = nc.dram_tensor("wq", [128, NKD_ * DPC], bf, kind="ExternalInput")
    wk = nc.dram_tensor("wk", [128, NKD_ * DPC], bf, kind="ExternalInput")
    wv = nc.dram_tensor("wv", [128, NKD_ * DPC], bf, kind="ExternalInput")
    wo = nc.dram_tensor("wo", [128, 4 * D], bf, kind="ExternalInput")
    bqd = nc.dram_tensor("bqd", [128, 4], f32, kind="ExternalInput")
    trid = nc.dram_tensor("trid", [128, 128], bf, kind="ExternalInput")
    y = nc.dram_tensor("y", [S, D], bf, kind="ExternalOutput")

    NKD = NKD_            # 8 contraction tiles for projections
    NST = S // 128        # 16 seq tiles
    NSB = NSB_            # 4 seq blocks (query blocks j)
    NHP = HPC // 2        # 4 head pairs / 128-wide col groups

    # staircase packing for diagonal k-tiles r=1..3: (packed col offset, width)
    STAIR = {1: (0, 384), 3: (384, 128), 2: (512, 256)}

    with tile.TileContext(nc) as tc:
        with (
            tc.tile_pool(name="consts", bufs=1) as consts,
            tc.tile_pool(name="wqp", bufs=NKD) as wqp,
            tc.tile_pool(name="wkp", bufs=NKD) as wkp,
            tc.tile_pool(name="wvp", bufs=NKD) as wvp,
            tc.tile_pool(name="wop", bufs=4) as wop,
            tc.tile_pool(name="xt", bufs=1) as xtp,
            tc.tile_pool(name="qt", bufs=NHP) as qtp,
            tc.tile_pool(name="kt", bufs=NHP) as ktp,
            tc.tile_pool(name="vp", bufs=NST) as vpool,
            tc.tile_pool(name="ex", bufs=3) as expool,
            tc.tile_pool(name="ot", bufs=NHP) as otp,
            tc.tile_pool(name="ys", bufs=2) as ysp,
            tc.tile_pool(name="rb", bufs=2) as rbp,
            tc.tile_pool(name="rc", bufs=2) as rcp,
            tc.tile_pool(name="ps", bufs=2, space="PSUM") as psp,
            tc.tile_pool(name="av", bufs=2, space="PSUM") as avp,
            tc.tile_pool(name="sc", bufs=2, space="PSUM") as scp,
        ):
            # (no PE warmup: HAM starts at full clock on hardware, and a
            # junk-matmul burst just trips the activity throttle early)
            trit = consts.tile([128, 128], bf, tag="tri")
            bqt = consts.tile([128, 4], f32, tag="bqt")
            onesc = consts.tile([1, 64], bf, tag="onesc")
            nc.gpsimd.memset(onesc[:], 1.0)


            # ---- input DMAs: one big descriptor per weight tensor / per
            # (x tensor, seq block) wave, all on the sync hw queue, in need
            # order. Consumers need the whole wave anyway (a projection group
            # reads all 8 kd slices of its sb).
            WW = NKD * 512    # 4096 cols per wave

            HW_ = WW // 2

            def dma_split(dst, src_ap, pieces):
                # several descriptors per first-phase panel: subtile deps let
                # the low-kd matmuls start as soon as the early pieces land
                pw = WW // pieces
                for i in range(pieces):
                    nc.sync.dma_start(dst[:, i * pw:(i + 1) * pw],
                                      src_ap[:, i * pw:(i + 1) * pw])

            def dma_halves(dst, src_ap):
                dma_split(dst, src_ap, 2)

            wvb = wvp.tile([128, WW], bf, name="wvb", tag="wv", bufs=1)
            dma_split(wvb, wv.ap(), 4)
            wvt = [wvb[:, kd * 512:(kd + 1) * 512] for kd in range(NKD)]

            def x_wave(src_t, prefix, tag, sb, halves=False):
                xt = xtp.tile([128, WW], bf, name=f"{prefix}{sb}", tag=tag,
                              bufs=1)
                src = src_t.ap()[:, sb * WW:(sb + 1) * WW]
                if halves:
                    dma_halves(xt, src)
                else:
                    nc.sync.dma_start(xt[:], src)
                return xt

            xtv_q = [[None] * NSB for _ in range(NKD)]
            xtq_q = [[None] * NSB for _ in range(NKD)]
            xtk_q = [[None] * NSB for _ in range(NKD)]

            def set_wave(qlist, big, sb):
                for kd in range(NKD):
                    qlist[kd][sb] = big[:, kd * 512:(kd + 1) * 512]

            # sb0 of everything first so V st0-3 / Q sb0 / K sb0 start early.
            # xtk waves share slots with older tiles whose readers finish well
            # before the k load's data is needed: xk0 is fresh, xk_sb (sb>=1)
            # reuses the xq_{sb-1} slot (read by the Q groups a phase earlier).
            xv0b = xtp.tile([128, WW], bf, name="xv0", tag="xv0", bufs=1)
            dma_split(xv0b, xtv.ap()[:, 0:WW], 4)
            set_wave(xtv_q, xv0b, 0)
            # small constants: after the two critical first waves, before the
            # rest (the triangle tile is first read by attend(0,0) ~25us in,
            # the q bias by the first Q eviction)
            nc.sync.dma_start(trit[:], trid.ap()[:])
            nc.sync.dma_start(bqt[:], bqd.ap()[:])
            wqb = wqp.tile([128, WW], bf, name="wqb", tag="wq", bufs=1)
            dma_halves(wqb, wq.ap())
            wqt = [wqb[:, kd * 512:(kd + 1) * 512] for kd in range(NKD)]
            set_wave(xtq_q, x_wave(xtq, "xq", "xq0", 0, halves=True), 0)
            wkb = wkp.tile([128, WW], bf, name="wkb", tag="wk", bufs=1)
            dma_halves(wkb, wk.ap())
            wkt = [wkb[:, kd * 512:(kd + 1) * 512] for kd in range(NKD)]
            set_wave(xtk_q, x_wave(xtk, "xk", "xk0", 0, halves=True), 0)
            # xk1 reuses the xq0 slot, xk3 reuses xq2; xk2 gets its own slot
            # (the xq1 readers it would wait on finish too late in phase 1)
            xk_tag = {1: "xq0", 2: "xk2", 3: "xq2"}
            for sb in range(1, NSB):
                set_wave(xtv_q, x_wave(xtv, "xv", f"xv{sb}", sb), sb)
                set_wave(xtq_q, x_wave(xtq, "xq", f"xq{sb}", sb), sb)
                set_wave(xtk_q, x_wave(xtk, "xk", xk_tag[sb], sb), sb)
                if sb == 1:
                    wob = wop.tile([128, 4 * D], bf, name="wob", tag="wo", bufs=1)
                    nc.sync.dma_start(wob[:], wo.ap()[:])
                    wot = [wob[:, hp * D:(hp + 1) * D] for hp in range(4)]

            # ---- V projection groups (natural layout, [8 heads x 65] incl.
            # ones column for the softmax denominator; no bias)
            vts = [vpool.tile([128, HPC * 65], bf, name=f"v{st}", tag="v")
                   for st in range(NST)]

            def v_group(st):
                def group():
                    ps = psp.tile([128, 512], f32, name="psv", tag="ps")
                    for kd in range(NKD):
                        nc.tensor.matmul(
                            ps[:],
                            xtv_q[kd][st // 4][:, (st % 4) * 128:(st % 4 + 1) * 128],
                            wvt[kd][:],
                            start=(kd == 0), stop=(kd == NKD - 1),
                        )
                    vt = vts[st]
                    v3 = vt[:].rearrange("p (h c) -> p h c", h=HPC, c=65)
                    nc.vector.tensor_copy(
                        v3[:, :, 0:64],
                        ps[:].rearrange("p (h c) -> p h c", h=HPC, c=64),
                    )
                    nc.gpsimd.memset(v3[:, :, 64:65], 1.0)
                return group

            # ---- QT / KT projection groups (transposed layout [cols, seq])
            qts, kts = [], []
            for pool, lst, nm in ((qtp, qts, "qt"), (ktp, kts, "kt")):
                for hp in range(NHP):
                    lst.append(pool.tile([128, S], bf, name=f"{nm}{hp}", tag=nm))

            def q_group(hp, sb):
                def group():
                    ps = psp.tile([128, 512], f32, name="psq", tag="ps")
                    for kd in range(NKD):
                        nc.tensor.matmul(
                            ps[:],
                            wqt[kd][:, hp * 128:(hp + 1) * 128],
                            xtq_q[kd][sb][:],
                            start=(kd == 0), stop=(kd == NKD - 1),
                        )
                    # bq rides the eviction as a per-partition ACT bias
                    nc.scalar.activation(
                        qts[hp][:, sb * 512:(sb + 1) * 512], ps[:], Ident,
                        bias=bqt[:, hp:hp + 1])
                return group

            def k_group(hp, sb):
                def group():
                    ps = psp.tile([128, 512], f32, name="psk", tag="ps")
                    for kd in range(NKD):
                        nc.tensor.matmul(
                            ps[:],
                            wkt[kd][:, hp * 128:(hp + 1) * 128],
                            xtk_q[kd][sb][:],
                            start=(kd == 0), stop=(kd == NKD - 1),
                        )
                    nc.vector.tensor_copy(kts[hp][:, sb * 512:(sb + 1) * 512], ps[:])
                return group

            ots = [otp.tile([128, S], bf, name=f"ot{i}", tag="ot") for i in range(NHP)]

            def yproj_group(st, eb):
                def group():
                    ps = psp.tile([128, 512], f32, name="psy", tag="ps")
                    for hp in range(NHP):
                        nc.tensor.matmul(
                            ps[:],
                            ots[hp][:, st * 128:(st + 1) * 128],
                            wot[hp][:, eb * 512:(eb + 1) * 512],
                            start=(hp == 0), stop=(hp == NHP - 1),
                        )
                    ys = ysp.tile([128, 512], bf, name="ys", tag="ys")
                    if (st + eb) % 2 == 0:
                        nc.vector.tensor_copy(ys[:], ps[:])
                    else:
                        nc.scalar.activation(ys[:], ps[:], Copy)
                    nc.sync.dma_start(
                        y.ap()[st * 128:(st + 1) * 128, eb * 512:(eb + 1) * 512],
                        ys[:],
                    )
                return group

            # ---- phase filler schedule (j-major attention). Within a phase,
            # fillers are ordered by DMA arrival: yproj (wo landed long ago)
            # first, then V/Q/K groups whose x quarters stream in during the
            # phase -- a popped filler whose input is still in flight stalls
            # the in-order PE queue.
            phase_fillers = {
                0: ([v_group(st) for st in range(4, 8)]
                    + [q_group(hp, 1) for hp in range(NHP)]
                    + [k_group(hp, 1) for hp in range(NHP)]),
                1: ([v_group(st) for st in range(8, 12)]
                    + [yproj_group(st, eb) for st in range(0, 4) for eb in range(2)]
                    + [q_group(hp, 2) for hp in range(NHP)]
                    + [k_group(hp, 2) for hp in range(NHP)]),
                2: ([v_group(st) for st in range(12, 16)]
                    + [yproj_group(st, eb) for st in range(4, 8) for eb in range(2)]
                    + [q_group(hp, 3) for hp in range(NHP)]
                    + [k_group(hp, 3) for hp in range(NHP)]),
                3: [yproj_group(st, eb) for st in range(8, 12) for eb in range(2)],
            }

            # ---- attention
            # pending[0] carries the delayed AV emit of the previous chunk --
            # including across attend/phase boundaries, so the PE never sits on
            # an exp tail: the next attend's scores interleave with it.
            # norm_tail[0] defers the cross-engine half of a normalize chain
            # one further flush point, so by the time its DVE ops reach the
            # queue head their sync-DMA/gpsimd inputs have already landed
            # (otherwise they head-of-line block the mask multiplies).
            pending = [None]
            norm_tail = [None]

            def flush_pending():
                if norm_tail[0] is not None:
                    t = norm_tail[0]
                    norm_tail[0] = None
                    t()
                if pending[0] is not None:
                    p = pending[0]
                    pending[0] = None
                    p()

            def attend(h, j, pop):
                hp, sub = h // 2, h % 2
                base = sub * 64
                qt_h = qts[hp][base:base + 64, :]
                kt_h = kts[hp][base:base + 64, :]
                av = avp.tile([128, 512], f32, name="av", tag="av")

                # chunks: pairs of full-width k-tiles 0..4j-1, then k-tile 4j
                # (the r0 diagonal) alone, then the packed staircase r=1..3
                chunks = [list(range(c0, min(c0 + 2, 4 * j)))
                          for c0 in range(0, 4 * j, 2)]
                chunks.append([4 * j])       # r0, full width, triangle at col 0
                chunks.append("stair")

                first_av = [True]

                def make_av_full(ex, kts_c, stop):
                    def emit():
                        for r, kti in enumerate(kts_c):
                            nc.tensor.matmul(
                                av[0:65, :],
                                vts[kti][:, h * 65:(h + 1) * 65],
                                ex[:, r * 512:(r + 1) * 512],
                                start=(first_av[0] and r == 0),
                                stop=(stop and r == len(kts_c) - 1),
                            )
                        first_av[0] = False
                    return emit

                def make_av_stair(ex):
                    def emit():
                        last = list(STAIR)[-1]
                        for r, (off, wid) in STAIR.items():
                            nc.tensor.matmul(
                                av[0:65, 128 * r:512],
                                vts[4 * j + r][:, h * 65:(h + 1) * 65],
                                ex[:, off:off + wid],
                                start=False, stop=(r == last),
                            )
                        # normalize, first half: evict av and kick off the
                        # [1,512]->[128,4] reshape (sync queue -- it is nearly
                        # idle; DVE reciprocal is ~6.5ns/element so the narrow
                        # reshape is what makes it cheap). The cross-engine
                        # tail is deferred one flush point via norm_tail so
                        # its DVE ops never head-of-line block the mask muls.
                        avs = ysp.tile([65, 512], f32, name="avs", tag="avs")
                        nc.vector.tensor_copy(avs[:], av[0:65, :])
                        if h == HPC - 1 and j == NSB - 1:
                            # very last attend: nothing left to hide the DMA/
                            # gpsimd round trips under, so take the direct
                            # path -- slow full-row DVE reciprocal (idle DVE),
                            # PE rank-1 broadcast into free av rows
                            rrow = rcp.tile([1, 512], bf, name="rrow",
                                            tag="rrow")
                            with nc.allow_low_precision(reason="bf16 1/d row"):
                                nc.vector.reciprocal(rrow[:], avs[64:65, :])
                            nc.tensor.matmul(av[64:128, :], onesc[:], rrow[:],
                                             start=True, stop=True)
                            nc.vector.tensor_mul(
                                ots[hp][base:base + 64, j * 512:(j + 1) * 512],
                                avs[0:64, :],
                                av[64:128, :],
                            )
                            return
                        rsh = rcp.tile([128, 4], f32, name="rsh", tag="rsh")
                        nc.sync.dma_start(rsh[:], avs[64:65, :])

                        def tail():
                            rr_t = rcp.tile([128, 4], f32, name="rr", tag="rr")
                            nc.vector.reciprocal(rr_t[:], rsh[:])
                            rrow = rcp.tile([1, 512], f32, name="rrow",
                                            tag="rrow")
                            nc.sync.dma_start(rrow[:], rr_t[:])
                            rb = rbp.tile([64, 512], f32, name="rb", tag="rb")
                            nc.gpsimd.partition_broadcast(rb[:], rrow[:],
                                                          channels=64)
                            nc.vector.tensor_mul(
                                ots[hp][base:base + 64, j * 512:(j + 1) * 512],
                                avs[0:64, :],
                                rb[:],
                            )
                        norm_tail[0] = tail
                    return emit

                for ci, ch in enumerate(chunks):
                    sc = scp.tile([128, 1024], f32, name="sc", tag="sc")
                    ex = expool.tile([128, 1024], bf, name="ex", tag="ex")
                    if ch == "stair":
                        for r, (off, wid) in STAIR.items():
                            nc.tensor.matmul(
                                sc[:, off:off + wid],
                                kt_h[:, (4 * j + r) * 128:(4 * j + r + 1) * 128],
                                qt_h[:, j * 512 + 128 * r:(j + 1) * 512],
                                start=True, stop=True,
                            )
                        nc.scalar.activation(ex[:, 0:768], sc[:, 0:768], Exp,
                                             scale=float(SCALE))
                        for r, (off, wid) in STAIR.items():
                            nc.vector.tensor_mul(
                                ex[:, off:off + 128],
                                ex[:, off:off + 128],
                                trit[:],
                            )
                        this_av = make_av_stair(ex)
                    else:
                        for r, kti in enumerate(ch):
                            nc.tensor.matmul(
                                sc[:, r * 512:(r + 1) * 512],
                                kt_h[:, kti * 128:(kti + 1) * 128],
                                qt_h[:, j * 512:(j + 1) * 512],
                                start=True, stop=True,
                            )
                        nw = len(ch) * 512
                        nc.scalar.activation(ex[:, 0:nw], sc[:, 0:nw], Exp,
                                             scale=float(SCALE))
                        if ch[-1] == 4 * j:   # r0 chunk: triangle at col 0
                            nc.vector.tensor_mul(
                                ex[:, (len(ch) - 1) * 512:(len(ch) - 1) * 512 + 128],
                                ex[:, (len(ch) - 1) * 512:(len(ch) - 1) * 512 + 128],
                                trit[:],
                            )
                        this_av = make_av_full(ex, ch, stop=False)
                    pop()
                    flush_pending()
                    pending[0] = this_av

            # upfront groups: V st0-3, Q sb0, K sb0
            for st in range(4):
                v_group(st)()
            for hp in range(NHP):
                q_group(hp, 0)()
            for hp in range(NHP):
                k_group(hp, 0)()

            for j in range(NSB):
                fillers = phase_fillers[j]
                nchunks = 8 * (j + 2)
                state = {"chunk": 0, "popped": 0}

                # last phase: hold ALL fillers for the drain -- per-chunk PE
                # work exceeds ACT there, so attends self-sustain, and the
                # final yproj tiles gate on the very last normalize chain
                hold = len(fillers) if j == NSB - 1 else 2

                def pop():
                    # hold fillers back so the phase-end drain always has PE
                    # work to cover the last attend's exp + normalize latency
                    # (more in the last phase: the final yproj tiles gate on
                    # the very last normalize)
                    state["chunk"] += 1
                    want = state["chunk"] * max(0, len(fillers) - hold) // nchunks
                    while state["popped"] < min(want, len(fillers)):
                        fillers[state["popped"]]()
                        state["popped"] += 1

                for h in range(HPC):
                    attend(h, j, pop)
                # drain remaining fillers, interleaving the two flushes (last
                # attend's stair AV, then its normalize tail) between them so
                # exp latency and the normalize chain hide under filler work
                for _ in range(2):
                    if state["popped"] < len(fillers):
                        fillers[state["popped"]]()
                        state["popped"] += 1
                    flush_pending()
                while state["popped"] < len(fillers):
                    fillers[state["popped"]]()
                    state["popped"] += 1

            flush_pending()
            flush_pending()
            for st in range(12, 16):
                for eb in range(2):
                    yproj_group(st, eb)()

    nc.compile()
    return nc


def _tri_mask():
    # tri[k, q] = 1 iff key k <= query q within a 128x128 diagonal block
    return np.triu(np.ones((128, 128), np.float32)).astype(BF16)


def _pack_x(xb):
    # [S, D] -> [128, (sb kd) 512]: xt_r[p, (sb*8+kd)*512+c] = x.T[kd*128+p, sb*512+c]
    xT = np.ascontiguousarray(xb.T)                      # [1024, 2048]
    return np.ascontiguousarray(
        xT.reshape(NKD_, 128, NSB_, 512).transpose(1, 2, 0, 3).reshape(128, -1)
    ).astype(BF16)


def _pack_w(Wh):
    # [D, 512] -> [128, kd-major 4096]
    return np.ascontiguousarray(
        Wh.reshape(NKD_, 128, DPC).transpose(1, 0, 2).reshape(128, -1)
    ).astype(BF16)


def _shard_inputs(q_in, k_in, v_in, Wq, bq, Wk, bk, Wv, bv, Wo, bo):
    tri = _tri_mask()
    in_maps = []
    for core in range(NCORES):
        b, g = core // 2, core % 2
        cs = slice(g * DPC, (g + 1) * DPC)
        in_maps.append({
            "xtq": _pack_x(q_in[b]),
            "xtk": _pack_x(k_in[b]),
            "xtv": _pack_x(v_in[b]),
            "wq": _pack_w(Wq[:, cs]),
            "wk": _pack_w(Wk[:, cs]),
            "wv": _pack_w(Wv[:, cs]),
            "wo": np.ascontiguousarray(
                Wo[cs, :].reshape(4, 128, D).transpose(1, 0, 2).reshape(128, -1)
            ).astype(BF16),
            "bqd": np.ascontiguousarray(
                bq[cs].reshape(4, 128).T).astype(np.float32),
            "trid": tri,
        })
    return in_maps


def kernel(q_in, k_in, v_in, Wq, bq, Wk, bk, Wv, bv, Wo, bo, _trace=False):
    from concourse.bass_utils import run_bass_kernel_spmd

    global _compiled
    if _compiled is None:
        _compiled = _build()

    args = [np.asarray(a, np.float32) for a in
            (q_in, k_in, v_in, Wq, bq, Wk, bk, Wv, bv, Wo, bo)]
    in_maps = _shard_inputs(*args)
    res = run_bass_kernel_spmd(
        _compiled, in_maps, core_ids=list(range(NCORES)), trace=_trace,
    )
    # bk cancels in softmax; bv commutes through (rows sum to 1): fold on host
    tail = (args[8].astype(np.float32) @ args[9].astype(np.float32)
            + args[10].astype(np.float32))
    out = np.empty((B, S, D), np.float32)
    for b in range(B):
        out[b] = (res.results[2 * b]["y"].astype(np.float32)
                  + res.results[2 * b + 1]["y"].astype(np.float32) + tail)
    if _trace:
        kernel.last_results = res
    return out


# revision 45
# speedup vs baseline: 1.4262x; 1.0316x over previous
"""Multi-head attention (B=4, S=2048, D=1024, H=16, causal) on 8 TRN2 NeuronCores.

Sharding: core i handles batch i//2 and head-group i%2 (8 heads / 512 projection
columns). Each core computes a partial output projection over its 512 rows of Wo;
the host sums the two partials per batch and adds (bv @ Wo + bo). No device
collectives.

v2 dataflow (bf16 matmuls, fp32 softmax), all per core:
  - j-major attention: for each query block j (512 queries), all 8 heads attend;
    projection groups and the j-1 output-projection tiles are woven in as PE
    filler between score/AV chunks.
  - Causal staircase: for (h, j), k-tiles 0..4j run full-width (512 queries) in
    2-k-tile PSUM chunks; the last three diagonal k-tiles r=1..3 only cover the
    un-masked query windows (384/128/256 wide) packed into one 768-col PSUM
    chunk, skipping the 6 fully-masked 128x128 blocks per (h, j). One wide exp
    per chunk; the only element-level masking left is four [128,128] triangle
    multiplies per (h, j) against a single shared triangle tile.
  - No bias matmuls: bk cancels in softmax (per-query constant), bv commutes
    through softmax (rows sum to 1) and is added on host as bv @ Wo, bq rides
    the ACT-engine Q eviction as a per-partition bias.
  - Denominator rides the AV matmul as a 65th V column; normalization uses a
    DMA-reshaped reciprocal ([1,512] -> [128,4]) and a GPSIMD partition
    broadcast, as in v1.
"""

import sys

for _p in ("/opt/trn_rl_repo",):
    if _p not in sys.path:
        sys.path.insert(0, _p)

import numpy as np
import ml_dtypes

BF16 = ml_dtypes.bfloat16

B, S, D = 4, 2048, 1024
H, HD = 16, 64
HPC = H // 2          # heads per core: 8
DPC = D // 2          # projection cols per core: 512
NCORES = 8
SCALE = 1.0 / np.sqrt(np.float32(HD))
NKD_ = D // 128       # 8 contraction tiles for projections
NSB_ = S // 512       # 4 seq blocks

_compiled = None


def _build():
    import concourse.bacc as bacc
    import concourse.mybir as mybir
    import concourse.tile as tile

    f32 = mybir.dt.float32
    bf = mybir.dt.bfloat16
    Exp = mybir.ActivationFunctionType.Exp
    Copy = mybir.ActivationFunctionType.Copy
    Ident = mybir.ActivationFunctionType.Identity

    nc = bacc.Bacc("TRN2", target_bir_lowering=False, debug=False)

    # host pre-packs everything into [128, *] panels so each tensor (or each
    # seq-block wave of an x tensor) loads with ONE dma descriptor: the sync
    # engine spends ~610ns generating each descriptor, so the v1 layout's 126
    # input descriptors serialized ~77us of input streaming.
    xtq = nc.dram_tensor("xtq", [128, NSB_ * NKD_ * 512], bf, kind="ExternalInput")
    xtk = nc.dram_tensor("xtk", [128, NSB_ * NKD_ * 512], bf, kind="ExternalInput")
    xtv = nc.dram_tensor("xtv", [128, NSB_ * NKD_ * 512], bf, kind="ExternalInput")
    wq = nc.dram_tensor("wq", [128, NKD_ * DPC], bf, kind="ExternalInput")
    wk = nc.dram_tensor("wk", [128, NKD_ * DPC], bf, kind="ExternalInput")
    wv = nc.dram_tensor("wv", [128, NKD_ * DPC], bf, kind="ExternalInput")
    wo = nc.dram_tensor("wo", [128, 4 * D], bf, kind="ExternalInput")
    bqd = nc.dram_tensor("bqd", [128, 4], f32, kind="ExternalInput")
    trid = nc.dram_tensor("trid", [128, 128], bf, kind="ExternalInput")
    y = nc.dram_tensor("y", [S, D], bf, kind="ExternalOutput")

    NKD = NKD_            # 8 contraction tiles for projections
    NST = S // 128        # 16 seq tiles
    NSB = NSB_            # 4 seq blocks (query blocks j)
    NHP = HPC // 2        # 4 head pairs / 128-wide col groups

    # staircase packing for diagonal k-tiles r=1..3: (packed col offset, width)
    STAIR = {1: (0, 384), 3: (384, 128), 2: (512, 256)}

    with tile.TileContext(nc) as tc:
        with (
            tc.tile_pool(name="consts", bufs=1) as consts,
            tc.tile_pool(name="wqp", bufs=NKD) as wqp,
            tc.tile_pool(name="wkp", bufs=NKD) as wkp,
            tc.tile_pool(name="wvp", bufs=NKD) as wvp,
            tc.tile_pool(name="wop", bufs=4) as wop,
            tc.tile_pool(name="xt", bufs=1) as xtp,
            tc.tile_pool(name="qt", bufs=NHP) as qtp,
            tc.tile_pool(name="kt", bufs=NHP) as ktp,
            tc.tile_pool(name="vp", bufs=NST) as vpool,
            tc.tile_pool(name="ex", bufs=3) as expool,
            tc.tile_pool(name="ot", bufs=NHP) as otp,
            tc.tile_pool(name="ys", bufs=2) as ysp,
            tc.tile_pool(name="rb", bufs=2) as rbp,
            tc.tile_pool(name="rc", bufs=2) as rcp,
            tc.tile_pool(name="ps", bufs=2, space="PSUM") as psp,
            tc.tile_pool(name="av", bufs=2, space="PSUM") as avp,
            tc.tile_pool(name="sc", bufs=2, space="PSUM") as scp,
        ):
            # (no PE warmup: HAM starts at full clock on hardware, and a
            # junk-matmul burst just trips the activity throttle early)
            trit = consts.tile([128, 128], bf, tag="tri")
            bqt = consts.tile([128, 4], f32, tag="bqt")
            onesc = consts.tile([1, 64], bf, tag="onesc")
            nc.gpsimd.memset(onesc[:], 1.0)


            # ---- input DMAs: one big descriptor per weight tensor / per
            # (x tensor, seq block) wave, all on the sync hw queue, in need
            # order. Consumers need the whole wave anyway (a projection group
            # reads all 8 kd slices of its sb).
            WW = NKD * 512    # 4096 cols per wave

            HW_ = WW // 2

            def dma_split(dst, src_ap, pieces):
                # several descriptors per first-phase panel: subtile deps let
                # the low-kd matmuls start as soon as the early pieces land
                pw = WW // pieces
                for i in range(pieces):
                    nc.sync.dma_start(dst[:, i * pw:(i + 1) * pw],
                                      src_ap[:, i * pw:(i + 1) * pw])

            def dma_halves(dst, src_ap):
                dma_split(dst, src_ap, 2)

            wvb = wvp.tile([128, WW], bf, name="wvb", tag="wv", bufs=1)
            dma_split(wvb, wv.ap(), 4)
            wvt = [wvb[:, kd * 512:(kd + 1) * 512] for kd in range(NKD)]

            def x_wave(src_t, prefix, tag, sb, halves=False):
                xt = xtp.tile([128, WW], bf, name=f"{prefix}{sb}", tag=tag,
                              bufs=1)
                src = src_t.ap()[:, sb * WW:(sb + 1) * WW]
                if halves:
                    dma_halves(xt, src)
                else:
                    nc.sync.dma_start(xt[:], src)
                return xt

            xtv_q = [[None] * NSB for _ in range(NKD)]
            xtq_q = [[None] * NSB for _ in range(NKD)]
            xtk_q = [[None] * NSB for _ in range(NKD)]

            def set_wave(qlist, big, sb):
                for kd in range(NKD):
                    qlist[kd][sb] = big[:, kd * 512:(kd + 1) * 512]

            # sb0 of everything first so V st0-3 / Q sb0 / K sb0 start early.
            # xtk waves share slots with older tiles whose readers finish well
            # before the k load's data is needed: xk0 is fresh, xk_sb (sb>=1)
            # reuses the xq_{sb-1} slot (read by the Q groups a phase earlier).
            xv0b = xtp.tile([128, WW], bf, name="xv0", tag="xv0", bufs=1)
            dma_split(xv0b, xtv.ap()[:, 0:WW], 4)
            set_wave(xtv_q, xv0b, 0)
            # small constants: after the two critical first waves, before the
            # rest (the triangle tile is first read by attend(0,0) ~25us in,
            # the q bias by the first Q eviction)
            nc.sync.dma_start(trit[:], trid.ap()[:])
            nc.sync.dma_start(bqt[:], bqd.ap()[:])
            wqb = wqp.tile([128, WW], bf, name="wqb", tag="wq", bufs=1)
            dma_halves(wqb, wq.ap())
            wqt = [wqb[:, kd * 512:(kd + 1) * 512] for kd in range(NKD)]
            set_wave(xtq_q, x_wave(xtq, "xq", "xq0", 0, halves=True), 0)
            wkb = wkp.tile([128, WW], bf, name="wkb", tag="wk", bufs=1)
            dma_halves(wkb, wk.ap())
            wkt = [wkb[:, kd * 512:(kd + 1) * 512] for kd in range(NKD)]
            set_wave(xtk_q, x_wave(xtk, "xk", "xk0", 0, halves=True), 0)
            # xk1 reuses the xq0 slot, xk3 reuses xq2; xk2 gets its own slot
            # (the xq1 readers it would wait on finish too late in phase 1)
            xk_tag = {1: "xq0", 2: "xk2", 3: "xq2"}
            for sb in range(1, NSB):
                set_wave(xtv_q, x_wave(xtv, "xv", f"xv{sb}", sb), sb)
                set_wave(xtq_q, x_wave(xtq, "xq", f"xq{sb}", sb), sb)
                set_wave(xtk_q, x_wave(xtk, "xk", xk_tag[sb], sb), sb)
                if sb == 1:
                    wob = wop.tile([128, 4 * D], bf, name="wob", tag="wo", bufs=1)
                    nc.sync.dma_start(wob[:], wo.ap()[:])
                    wot = [wob[:, hp * D:(hp + 1) * D] for hp in range(4)]

            # ---- V projection groups (natural layout, [8 heads x 65] incl.
            # ones column for the softmax denominator; no bias)
            vts = [vpool.tile([128, HPC * 65], bf, name=f"v{st}", tag="v")
                   for st in range(NST)]

            def v_group(st):
                def group():
                    ps = psp.tile([128, 512], f32, name="psv", tag="ps")
                    for kd in range(NKD):
                        nc.tensor.matmul(
                            ps[:],
                            xtv_q[kd][st // 4][:, (st % 4) * 128:(st % 4 + 1) * 128],
                            wvt[kd][:],
                            start=(kd == 0), stop=(kd == NKD - 1),
                        )
                    vt = vts[st]
                    v3 = vt[:].rearrange("p (h c) -> p h c", h=HPC, c=65)
                    nc.vector.tensor_copy(
                        v3[:, :, 0:64],
                        ps[:].rearrange("p (h c) -> p h c", h=HPC, c=64),
                    )
                    nc.gpsimd.memset(v3[:, :, 64:65], 1.0)
                return group

            # ---- QT / KT projection groups (transposed layout [cols, seq])
            qts, kts = [], []
            for pool, lst, nm in ((qtp, qts, "qt"), (ktp, kts, "kt")):
                for hp in range(NHP):
                    lst.append(pool.tile([128, S], bf, name=f"{nm}{hp}", tag=nm))

            def q_group(hp, sb):
                def group():
                    ps = psp.tile([128, 512], f32, name="psq", tag="ps")
                    for kd in range(NKD):
                        nc.tensor.matmul(
                            ps[:],
                            wqt[kd][:, hp * 128:(hp + 1) * 128],
                            xtq_q[kd][sb][:],
                            start=(kd == 0), stop=(kd == NKD - 1),
                        )
                    # bq rides the eviction as a per-partition ACT bias
                    nc.scalar.activation(
                        qts[hp][:, sb * 512:(sb + 1) * 512], ps[:], Ident,
                        bias=bqt[:, hp:hp + 1])
                return group

            def k_group(hp, sb):
                def group():
                    ps = psp.tile([128, 512], f32, name="psk", tag="ps")
                    for kd in range(NKD):
                        nc.tensor.matmul(
                            ps[:],
                            wkt[kd][:, hp * 128:(hp + 1) * 128],
                            xtk_q[kd][sb][:],
                            start=(kd == 0), stop=(kd == NKD - 1),
                        )
                    nc.vector.tensor_copy(kts[hp][:, sb * 512:(sb + 1) * 512], ps[:])
                return group

            ots = [otp.tile([128, S], bf, name=f"ot{i}", tag="ot") for i in range(NHP)]

            def yproj_group(st, eb):
                def group():
                    ps = psp.tile([128, 512], f32, name="psy", tag="ps")
                    for hp in range(NHP):
                        nc.tensor.matmul(
                            ps[:],
                            ots[hp][:, st * 128:(st + 1) * 128],
                            wot[hp][:, eb * 512:(eb + 1) * 512],
                            start=(hp == 0), stop=(hp == NHP - 1),
                        )
                    ys = ysp.tile([128, 512], bf, name="ys", tag="ys")
                    if (st + eb) % 2 == 0:
                        nc.vector.tensor_copy(ys[:], ps[:])
                    else:
                        nc.scalar.activation(ys[:], ps[:], Copy)
                    nc.sync.dma_start(
                        y.ap()[st * 128:(st + 1) * 128, eb * 512:(eb + 1) * 512],
                        ys[:],
                    )
                return group

            # ---- phase filler schedule (j-major attention). Within a phase,
            # fillers are ordered by DMA arrival: yproj (wo landed long ago)
            # first, then V/Q/K groups whose x quarters stream in during the
            # phase -- a popped filler whose input is still in flight stalls
            # the in-order PE queue.
            phase_fillers = {
                0: ([v_group(st) for st in range(4, 8)]
                    + [q_group(hp, 1) for hp in range(NHP)]
                    + [k_group(hp, 1) for hp in range(NHP)]),
                1: ([v_group(st) for st in range(8, 12)]
                    + [q_group(hp, 2) for hp in range(NHP)]
                    + [k_group(hp, 2) for hp in range(NHP)]
                    + [yproj_group(st, eb) for st in range(0, 4) for eb in range(2)]),
                2: ([v_group(st) for st in range(12, 16)]
                    + [q_group(hp, 3) for hp in range(NHP)]
                    + [k_group(hp, 3) for hp in range(NHP)]
                    + [yproj_group(st, eb) for st in range(4, 8) for eb in range(2)]),
                3: [yproj_group(st, eb) for st in range(8, 12) for eb in range(2)],
            }

            # ---- attention
            # pending[0] carries the delayed AV emit of the previous chunk --
            # including across attend/phase boundaries, so the PE never sits on
            # an exp tail: the next attend's scores interleave with it.
            # norm_tail[0] defers the cross-engine half of a normalize chain
            # one further flush point, so by the time its DVE ops reach the
            # queue head their sync-DMA/gpsimd inputs have already landed
            # (otherwise they head-of-line block the mask multiplies).
            pending = [None]
            norm_tail = [None]

            def flush_pending():
                if norm_tail[0] is not None:
                    t = norm_tail[0]
                    norm_tail[0] = None
                    t()
                if pending[0] is not None:
                    p = pending[0]
                    pending[0] = None
                    p()

            def attend(h, j, pop):
                hp, sub = h // 2, h % 2
                base = sub * 64
                qt_h = qts[hp][base:base + 64, :]
                kt_h = kts[hp][base:base + 64, :]
                av = avp.tile([128, 512], f32, name="av", tag="av")

                # chunks: pairs of full-width k-tiles 0..4j-1, then k-tile 4j
                # (the r0 diagonal) alone, then the packed staircase r=1..3
                chunks = [list(range(c0, min(c0 + 2, 4 * j)))
                          for c0 in range(0, 4 * j, 2)]
                chunks.append([4 * j])       # r0, full width, triangle at col 0
                chunks.append("stair")

                first_av = [True]

                def make_av_full(ex, kts_c, stop):
                    def emit():
                        for r, kti in enumerate(kts_c):
                            nc.tensor.matmul(
                                av[0:65, :],
                                vts[kti][:, h * 65:(h + 1) * 65],
                                ex[:, r * 512:(r + 1) * 512],
                                start=(first_av[0] and r == 0),
                                stop=(stop and r == len(kts_c) - 1),
                            )
                        first_av[0] = False
                    return emit

                def make_av_stair(ex):
                    def emit():
                        last = list(STAIR)[-1]
                        for r, (off, wid) in STAIR.items():
                            nc.tensor.matmul(
                                av[0:65, 128 * r:512],
                                vts[4 * j + r][:, h * 65:(h + 1) * 65],
                                ex[:, off:off + wid],
                                start=False, stop=(r == last),
                            )
                        # normalize, first half: evict av and kick off the
                        # [1,512]->[128,4] reshape (sync queue -- it is nearly
                        # idle; DVE reciprocal is ~6.5ns/element so the narrow
                        # reshape is what makes it cheap). The cross-engine
                        # tail is deferred one flush point via norm_tail so
                        # its DVE ops never head-of-line block the mask muls.
                        avs = ysp.tile([65, 512], f32, name="avs", tag="avs")
                        nc.vector.tensor_copy(avs[:], av[0:65, :])
                        if h == HPC - 1 and j == NSB - 1:
                            # very last attend: nothing left to hide the DMA/
                            # gpsimd round trips under, so take the direct
                            # path -- slow full-row DVE reciprocal (idle DVE),
                            # PE rank-1 broadcast into free av rows
                            rrow = rcp.tile([1, 512], bf, name="rrow",
                                            tag="rrow")
                            with nc.allow_low_precision(reason="bf16 1/d row"):
                                nc.vector.reciprocal(rrow[:], avs[64:65, :])
                            nc.tensor.matmul(av[64:128, :], onesc[:], rrow[:],
                                             start=True, stop=True)
                            nc.vector.tensor_mul(
                                ots[hp][base:base + 64, j * 512:(j + 1) * 512],
                                avs[0:64, :],
                                av[64:128, :],
                            )
                            return
                        rsh = rcp.tile([128, 4], f32, name="rsh", tag="rsh")
                        nc.sync.dma_start(rsh[:], avs[64:65, :])

                        def tail():
                            rr_t = rcp.tile([128, 4], f32, name="rr", tag="rr")
                            nc.vector.reciprocal(rr_t[:], rsh[:])
                            rrow = rcp.tile([1, 512], f32, name="rrow",
                                            tag="rrow")
                            nc.sync.dma_start(rrow[:], rr_t[:])
                            rb = rbp.tile([64, 512], f32, name="rb", tag="rb")
                            nc.gpsimd.partition_broadcast(rb[:], rrow[:],
                                                          channels=64)
                            nc.vector.tensor_mul(
                                ots[hp][base:base + 64, j * 512:(j + 1) * 512],
                                avs[0:64, :],
                                rb[:],
                            )
                        norm_tail[0] = tail
                    return emit

                for ci, ch in enumerate(chunks):
                    sc = scp.tile([128, 1024], f32, name="sc", tag="sc")
                    ex = expool.tile([128, 1024], bf, name="ex", tag="ex")
                    if ch == "stair":
                        for r, (off, wid) in STAIR.items():
                            nc.tensor.matmul(
                                sc[:, off:off + wid],
                                kt_h[:, (4 * j + r) * 128:(4 * j + r + 1) * 128],
                                qt_h[:, j * 512 + 128 * r:(j + 1) * 512],
                                start=True, stop=True,
                            )
                        nc.scalar.activation(ex[:, 0:768], sc[:, 0:768], Exp,
                                             scale=float(SCALE))
                        for r, (off, wid) in STAIR.items():
                            nc.vector.tensor_mul(
                                ex[:, off:off + 128],
                                ex[:, off:off + 128],
                                trit[:],
                            )
                        this_av = make_av_stair(ex)
                    else:
                        for r, kti in enumerate(ch):
                            nc.tensor.matmul(
                                sc[:, r * 512:(r + 1) * 512],
                                kt_h[:, kti * 128:(kti + 1) * 128],
                                qt_h[:, j * 512:(j + 1) * 512],
                                start=True, stop=True,
                            )
                        nw = len(ch) * 512
                        nc.scalar.activation(ex[:, 0:nw], sc[:, 0:nw], Exp,
                                             scale=float(SCALE))
                        if ch[-1] == 4 * j:   # r0 chunk: triangle at col 0
                            nc.vector.tensor_mul(
                                ex[:, (len(ch) - 1) * 512:(len(ch) - 1) * 512 + 128],
                                ex[:, (len(ch) - 1) * 512:(len(ch) - 1) * 512 + 128],
                                trit[:],
                            )
                        this_av = make_av_full(ex, ch, stop=False)
                    pop()
                    flush_pending()
                    pending[0] = this_av

            # upfront groups: V st0-3, Q sb0, K sb0
            for st in range(4):
                v_group(st)()
            for hp in range(NHP):
                q_group(hp, 0)()
            for hp in range(NHP):
                k_group(hp, 0)()

            for j in range(NSB):
                fillers = phase_fillers[j]
                nchunks = 8 * (j + 2)
                state = {"chunk": 0, "popped": 0}

                # last phase: hold ALL fillers for the drain -- per-chunk PE
                # work exceeds ACT there, so attends self-sustain, and the
                # final yproj tiles gate on the very last normalize chain
                hold = len(fillers) if j == NSB - 1 else 2

                def pop():
                    # hold fillers back so the phase-end drain always has PE
                    # work to cover the last attend's exp + normalize latency
                    # (more in the last phase: the final yproj tiles gate on
                    # the very last normalize)
                    state["chunk"] += 1
                    want = state["chunk"] * max(0, len(fillers) - hold) // nchunks
                    while state["popped"] < min(want, len(fillers)):
                        fillers[state["popped"]]()
                        state["popped"] += 1

                for h in range(HPC):
                    attend(h, j, pop)
                # drain remaining fillers, interleaving the two flushes (last
                # attend's stair AV, then its normalize tail) between them so
                # exp latency and the normalize chain hide under filler work
                for _ in range(2):
                    if state["popped"] < len(fillers):
                        fillers[state["popped"]]()
                        state["popped"] += 1
                    flush_pending()
                while state["popped"] < len(fillers):
                    fillers[state["popped"]]()
                    state["popped"] += 1

            flush_pending()
            flush_pending()
            for st in range(12, 16):
                for eb in range(2):
                    yproj_group(st, eb)()

    nc.compile()
    return nc


def _tri_mask():
    # tri[k, q] = 1 iff key k <= query q within a 128x128 diagonal block
    return np.triu(np.ones((128, 128), np.float32)).astype(BF16)


def _pack_x(xb):
    # [S, D] -> [128, (sb kd) 512]: xt_r[p, (sb*8+kd)*512+c] = x.T[kd*128+p, sb*512+c]
    xT = np.ascontiguousarray(xb.T)                      # [1024, 2048]
    return np.ascontiguousarray(
        xT.reshape(NKD_, 128, NSB_, 512).transpose(1, 2, 0, 3).reshape(128, -1)
    ).astype(BF16)


def _pack_w(Wh):
    # [D, 512] -> [128, kd-major 4096]
    return np.ascontiguousarray(
        Wh.reshape(NKD_, 128, DPC).transpose(1, 0, 2).reshape(128, -1)
    ).astype(BF16)


def _shard_inputs(q_in, k_in, v_in, Wq, bq, Wk, bk, Wv, bv, Wo, bo):
    tri = _tri_mask()
    in_maps = []
    for core in range(NCORES):
        b, g = core // 2, core % 2
        cs = slice(g * DPC, (g + 1) * DPC)
        in_maps.append({
            "xtq": _pack_x(q_in[b]),
            "xtk": _pack_x(k_in[b]),
            "xtv": _pack_x(v_in[b]),
            "wq": _pack_w(Wq[:, cs]),
            "wk": _pack_w(Wk[:, cs]),
            "wv": _pack_w(Wv[:, cs]),
            "wo": np.ascontiguousarray(
                Wo[cs, :].reshape(4, 128, D).transpose(1, 0, 2).reshape(128, -1)
            ).astype(BF16),
            "bqd": np.ascontiguousarray(
                bq[cs].reshape(4, 128).T).astype(np.float32),
            "trid": tri,
        })
    return in_maps


def kernel(q_in, k_in, v_in, Wq, bq, Wk, bk, Wv, bv, Wo, bo, _trace=False):
    from concourse.bass_utils import run_bass_kernel_spmd

    global _compiled
    if _compiled is None:
        _compiled = _build()

    args = [np.asarray(a, np.float32) for a in
            (q_in, k_in, v_in, Wq, bq, Wk, bk, Wv, bv, Wo, bo)]
    in_maps = _shard_inputs(*args)
    res = run_bass_kernel_spmd(
        _compiled, in_maps, core_ids=list(range(NCORES)), trace=_trace,
    )
    # bk cancels in softmax; bv commutes through (rows sum to 1): fold on host
    tail = (args[8].astype(np.float32) @ args[9].astype(np.float32)
            + args[10].astype(np.float32))
    out = np.empty((B, S, D), np.float32)
    for b in range(B):
        out[b] = (res.results[2 * b]["y"].astype(np.float32)
                  + res.results[2 * b + 1]["y"].astype(np.float32) + tail)
    if _trace:
        kernel.last_results = res
    return out
